# revision 2
# baseline (speedup 1.0000x reference)
"""Trainium2 Bass kernel for a 4-layer transformer (B=2,S=1024,D=1024,H=16,F=4096,V=32000).

Strategy (8 NeuronCores): sequence-parallel layers (256 tokens/core, weights
replicated, streamed bf16), feature-major activations, per-layer merged K+V
AllGather within each batch's 4-core group, vocab-sharded LM head.

v2 changes vs baseline:
 - Weights streamed as per-output-tile bundles ([128, 8dt, 128] host-packed)
   through small rotating SBUF slots instead of 8 resident 1MB tiles; fc2 is
   consumed ft-outer right behind fc1+gelu, accumulating into 8 PSUM banks.
 - LN scale/bias folded into the following weights on host; device LN is pure
   standardize with DVE Newton-rsqrt (ACT only runs Exp + Gelu).
 - All small per-layer params packed into one [128, 72] tensor (one DMA).
 - K and V gathered in ONE AllGather per layer; gathered K/V streamed per-j.
 - Attention scores packed 2-per-PSUM-bank -> exp on [128,512] tiles.
 - PSUM evacuations/bias-adds moved from ACT to DVE.
"""
import numpy as np
import ml_dtypes

import concourse.bass as bass
import concourse.bacc as bacc
import concourse.mybir as mybir
import concourse.tile as tile
from concourse import bass_utils
from concourse.masks import make_identity

B, S, D, H, L, F, V = 2, 1024, 1024, 16, 4, 4096, 32000
DH = D // H          # 64
NCORES = 8
T = (B * S) // NCORES  # 256 tokens per core
NT = B * S             # 2048
VS = V // NCORES       # 4000
VSP = 4096             # padded vocab shard
P = 128
ND = D // P            # 8 d-tiles
NFT = F // P           # 32 fc1 f-tiles
NPAR = 72              # packed params: qkvb[0:24] projb[24:32] fc1b[32:64] fc2b[64:72]
KSZ = 8 * P * 256          # k elems in merged AG buffer
VSZ = 256 * 16 * 65        # v elems ([2 th, 128 p, 16h*65])
KVTOT = KSZ + VSZ

f32 = mybir.dt.float32
bf16 = mybir.dt.bfloat16
i32 = mybir.dt.int32
u32 = mybir.dt.uint32
AF = mybir.ActivationFunctionType
OP = mybir.AluOpType

RSQRT_MAGIC_P1 = 0x5F3759E0  # 0x5f3759df + 1 (two's-complement sub via xor+add)


def _ln(nc, ps, act, rows, cons, x_sb, out_h):
    """Standardize: x_sb [128, 8, 256] f32 -> out_h [128, 8, 256] bf16.

    (x - mu) * rsqrt(var + eps); LN scale/bias are folded into the next
    weight matrix on the host. Stats via paired N=512 ones-matmuls; rsqrt
    via DVE bit-hack seed + 2 Newton steps (ACT tables stay on exp/gelu)."""
    stat_x = ps.tile([1, 512], f32, tag="att_s", bufs=4)
    stat_q = ps.tile([1, 512], f32, tag="att_s", bufs=4)
    for dp in range(4):
        xp = x_sb[:, 2 * dp:2 * dp + 2, :].rearrange("p a b -> p (a b)")
        sq = act.tile([P, 512], f32, tag="sq", bufs=2)
        nc.vector.tensor_mul(sq[:], xp, xp)
        nc.tensor.matmul(stat_x[:], lhsT=cons.ones_col_f32[:], rhs=xp,
                         start=(dp == 0), stop=(dp == 3))
        nc.tensor.matmul(stat_q[:], lhsT=cons.ones_col_f32[:], rhs=sq[:],
                         start=(dp == 0), stop=(dp == 3))
    murow = rows.tile([1, 512], f32, tag="row")  # [mu | invstd]
    sxrow = rows.tile([1, 512], f32, tag="row")
    nc.vector.tensor_copy(sxrow[:], stat_x[:])
    sqrow = rows.tile([1, 512], f32, tag="row")
    nc.vector.tensor_copy(sqrow[:], stat_q[:])
    sumx = rows.tile([1, 256], f32, tag="row")
    nc.vector.tensor_add(sumx[:], sxrow[:, 0:256], sxrow[:, 256:512])
    nc.vector.tensor_scalar(murow[:, 0:256], sumx[:], 1.0 / D, None, OP.mult)
    msq = rows.tile([1, 256], f32, tag="row")
    nc.vector.tensor_add(msq[:], sqrow[:, 0:256], sqrow[:, 256:512])
    nc.vector.tensor_scalar(msq[:], msq[:], 1.0 / D, 1e-5, OP.mult, OP.add)
    mu2 = rows.tile([1, 256], f32, tag="row")
    nc.vector.tensor_mul(mu2[:], murow[:, 0:256], murow[:, 0:256])
    vare = rows.tile([1, 256], f32, tag="row")
    nc.vector.tensor_sub(vare[:], msq[:], mu2[:])
    std = rows.tile([1, 256], f32, tag="row")
    nc.scalar.activation(std[:], vare[:], AF.Sqrt)
    nc.vector.reciprocal(murow[:, 256:512], std[:])
    bc = act.tile([P, 512], f32, tag="lnbc", bufs=2)
    nc.gpsimd.partition_broadcast(bc[:], murow[:], channels=P)
    for dt in range(ND):
        t = act.tile([P, 256], f32, tag="lnt", bufs=3)
        nc.vector.tensor_sub(t[:], x_sb[:, dt, :], bc[:, 0:256])
        nc.vector.tensor_mul(out_h[:, dt, :], t[:], bc[:, 256:512])


class _Cons:
    pass


def build(n_layers=L, single=False):
    """single=True: 1-core variant with collectives replaced by local DMA
    copies (for TimelineSim cost-model analysis only — wrong numerics)."""
    nc = bacc.Bacc("TRN2", target_bir_lowering=False, debug=False,
                   num_devices=1 if single else NCORES)

    ids = nc.dram_tensor("ids", [T], i32, kind="ExternalInput").ap()
    pos = nc.dram_tensor("pos", [T, D], f32, kind="ExternalInput").ap()
    embed_w = nc.dram_tensor("embed_w", [V, D], f32, kind="ExternalInput").ap()
    # qk bundles: [L, 16, 128, 8dt*128] — k fts first (8), then q fts (8)
    qk_wb = nc.dram_tensor("qk_wb", [L, 16, P, ND * P], bf16, kind="ExternalInput").ap()
    # v weight stream: [L, 2nb, 8dt, 128, 512]
    v_wb = nc.dram_tensor("v_wb", [L, 2, ND, P, 512], bf16, kind="ExternalInput").ap()
    # proj bundles: [L, 8do, 128, 8dt*128]
    pj_wb = nc.dram_tensor("pj_wb", [L, ND, P, ND * P], bf16, kind="ExternalInput").ap()
    # fc1 bundles: [L, 32ft, 128, 8dt*128]
    f1_wb = nc.dram_tensor("f1_wb", [L, NFT, P, ND * P], bf16, kind="ExternalInput").ap()
    # fc2 per-ft rows: [L, 32ft, 128, 1024]
    f2_wb = nc.dram_tensor("f2_wb", [L, NFT, P, D], bf16, kind="ExternalInput").ap()
    parms = nc.dram_tensor("parms", [L, P, NPAR], f32, kind="ExternalInput").ap()
    lm_wT = nc.dram_tensor("lm_wT", [D, VSP], bf16, kind="ExternalInput").ap()
    lm_b = nc.dram_tensor("lm_b", [VSP], f32, kind="ExternalInput").ap()
    out_tok = nc.dram_tensor("out_tok", [NT, VSP], f32, kind="ExternalOutput").ap()

    kv_groups = [[0, 1, 2, 3], [4, 5, 6, 7]]
    all_group = [list(range(NCORES))]

    with tile.TileContext(nc) as tc:
        with (
            tc.tile_pool(name="consp", bufs=1) as consp,
            tc.tile_pool(name="wlm", bufs=8) as wlm,
            tc.tile_pool(name="wstr", bufs=1) as wstr,
            tc.tile_pool(name="rows", bufs=6) as rows,
            tc.tile_pool(name="par", bufs=2) as par,
            tc.tile_pool(name="dram", bufs=1, space="DRAM") as dram,
        ):
            act = tc.alloc_tile_pool(name="act", bufs=1)
            ps = tc.alloc_tile_pool(name="ps", bufs=2, space="PSUM")
            cons = _Cons()
            ident = consp.tile([P, P], f32)
            make_identity(nc, ident)
            ident_bf = consp.tile([P, P], bf16)
            nc.vector.tensor_copy(ident_bf[:], ident[:])
            ones_col_f32 = consp.tile([P, 1], f32)
            nc.vector.memset(ones_col_f32[:], 1.0)
            cons.ones_col_f32 = ones_col_f32

            x_sb = consp.tile([P, ND, 256], f32)  # residual, feature-major

            # per-core group-rank registers for own-block-skipping dynamic DMAs
            seng = nc.sync
            pid = seng.partition_id()
            rgrp = seng.alloc_register("grp_rank")
            seng.reg_alu(rgrp, pid, 3, OP.bitwise_and)
            grp_rank = seng.snap(rgrp, donate=True, min_val=0, max_val=3)
            oth_ranks = []
            for i in range(3):
                ra = seng.alloc_register(f"oth{i}a")
                seng.reg_alu(ra, grp_rank, i + 1, OP.add)
                rb = seng.alloc_register(f"oth{i}b")
                seng.reg_alu(rb, ra, 3, OP.bitwise_and)
                oth_ranks.append(seng.snap(rb, donate=True, min_val=0, max_val=3))

            # ---------------- embedding ----------------
            for tc2 in range(2):
                ids_sb = par.tile([P, 1], i32, tag="ids")
                nc.sync.dma_start(ids_sb[:], ids[tc2 * P:(tc2 + 1) * P, None])
                gat = act.tile([P, D], f32, tag="emb", bufs=2, name=f"gat{tc2}")
                nc.gpsimd.indirect_dma_start(
                    out=gat[:], out_offset=None, in_=embed_w[:],
                    in_offset=bass.IndirectOffsetOnAxis(ap=ids_sb[:, :1], axis=0))
                pos_sb = act.tile([P, D], f32, tag="emb", bufs=2, name=f"pos{tc2}")
                nc.sync.dma_start(pos_sb[:], pos[tc2 * P:(tc2 + 1) * P, :])
                nc.vector.tensor_add(gat[:], gat[:], pos_sb[:])
                for dt in range(ND):
                    tp = ps.tile([P, P], f32, tag="att_s", bufs=4)
                    nc.tensor.transpose(tp[:], gat[:, dt * P:(dt + 1) * P], ident[:])
                    nc.vector.tensor_copy(x_sb[:, dt, tc2 * P:(tc2 + 1) * P], tp[:])

            # ---------------- layers ----------------
            for l in range(n_layers):
                parms_t = par.tile([P, NPAR], f32, tag="lnp")
                nc.sync.dma_start(parms_t[:], parms[l])

                h_sb = act.tile([P, ND, 256], bf16, tag="h", bufs=2)
                _ln(nc, ps, act, rows, cons, x_sb, h_sb)

                q_all = act.tile([P, 8, 256], bf16, tag="q")
                k_loc = act.tile([P, 8, 256], bf16, tag="kloc")
                # K first so the AllGather can start as early as possible
                for kf in range(8):
                    wb = wstr.tile([P, ND, P], bf16, tag="wb", bufs=6,
                                   name=f"wk{l}_{kf}")
                    nc.sync.dma_start(
                        wb[:].rearrange("p a b -> p (a b)"), qk_wb[l, kf])
                    acc = ps.tile([P, 256], f32, tag="acc")
                    for dt in range(ND):
                        nc.tensor.matmul(acc[:], lhsT=wb[:, dt, :],
                                         rhs=h_sb[:, dt, :],
                                         start=(dt == 0), stop=(dt == ND - 1))
                    nc.vector.tensor_scalar_add(k_loc[:, kf, :], acc[:],
                                                parms_t[:, 8 + kf:8 + kf + 1])

                # V (token-major, per head 65 cols = [v_h | 1])
                v_loc = act.tile([P, 2, 16 * 65], bf16, tag="vloc")
                v_loc_h = v_loc.rearrange("p c (h g) -> p c h g", h=16, g=65)
                for nb in range(2):
                    accs = [ps.tile([P, 512], f32, tag="av", bufs=2,
                                    name=f"vacc{l}_{nb}_{tc2}")
                            for tc2 in range(2)]
                    for dt in range(ND):
                        wv = wstr.tile([P, 512], bf16, tag="wv", bufs=4,
                                       name=f"wv{l}_{nb}_{dt}")
                        nc.sync.dma_start(wv[:], v_wb[l, nb, dt])
                        for tc2 in range(2):
                            nc.tensor.matmul(
                                accs[tc2][:],
                                lhsT=h_sb[:, dt, tc2 * P:(tc2 + 1) * P],
                                rhs=wv[:],
                                start=(dt == 0), stop=(dt == ND - 1))
                    for tc2 in range(2):
                        nc.vector.tensor_copy(
                            v_loc_h[:, tc2, nb * 8:(nb + 1) * 8, 0:64],
                            accs[tc2][:].rearrange("p (h g) -> p h g", h=8))
                for tc2 in range(2):
                    nc.vector.memset(v_loc_h[:, tc2, :, 64:65], 1.0)

                # merged K+V bounce + single AllGather per layer
                kv_in = dram.tile([KVTOT], bf16, tag="kvin", name=f"kvin{l}")
                kv_out = dram.tile([4, KVTOT], bf16, tag="kvout", name=f"kvout{l}")
                nc.sync.dma_start(
                    kv_in[0:KSZ].rearrange("(f p t) -> p f t", f=8, p=P), k_loc[:])
                for tc2 in range(2):
                    nc.sync.dma_start(
                        kv_in[KSZ + tc2 * P * 1040:KSZ + (tc2 + 1) * P * 1040]
                        .rearrange("(p f) -> p f", p=P),
                        v_loc[:, tc2, :])
                if single:
                    nc.sync.dma_start(kv_out[0], kv_in[:])
                else:
                    nc.gpsimd.collective_compute(
                        "AllGather", OP.bypass, replica_groups=kv_groups,
                        ins=[kv_in.opt()], outs=[kv_out.opt()])

                # Q projections (overlap the AllGather)
                for qf in range(8):
                    wb = wstr.tile([P, ND, P], bf16, tag="wb", bufs=6,
                                   name=f"wq{l}_{qf}")
                    nc.sync.dma_start(
                        wb[:].rearrange("p a b -> p (a b)"), qk_wb[l, 8 + qf])
                    acc = ps.tile([P, 256], f32, tag="acc")
                    for dt in range(ND):
                        nc.tensor.matmul(acc[:], lhsT=wb[:, dt, :],
                                         rhs=h_sb[:, dt, :],
                                         start=(dt == 0), stop=(dt == ND - 1))
                    nc.vector.tensor_scalar_add(q_all[:, qf, :], acc[:],
                                                parms_t[:, qf:qf + 1])

                o_sb = act.tile([P, ND, 256], bf16, tag="o")
                scale = 1.0 / np.sqrt(DH)

                # Pass 1 (pre-AllGather): attention over this core's OWN 256
                # k-tokens; both k-chunks share one PSUM bank -> one exp each.
                o_own = {}
                for j in range(8):
                    for hh in range(2):
                        h_idx = 2 * j + hh
                        base = hh * 64
                        sps = ps.tile([P, 512], f32, tag="att_s", bufs=4,
                                      name=f"spp{l}_{j}_{hh}")
                        for c in range(2):
                            nc.tensor.matmul(
                                sps[:, c * 256:(c + 1) * 256],
                                lhsT=k_loc[base:base + 64, j, c * P:(c + 1) * P],
                                rhs=q_all[base:base + 64, j, :],
                                start=(c == 0), stop=True)
                        e = act.tile([P, 512], bf16, tag="e", bufs=4,
                                     name=f"ep{l}_{j}_{hh}")
                        nc.scalar.activation(e[:], sps[:], AF.Exp, scale=scale)
                        avp = ps.tile([P, 512], f32, tag="av", bufs=2,
                                      name=f"avp{l}_{j}_{hh}")
                        for c in range(2):
                            nc.tensor.matmul(
                                avp[0:65, 0:256],
                                lhsT=v_loc_h[:, c, h_idx, :],
                                rhs=e[:, c * 256:(c + 1) * 256],
                                start=(c == 0), stop=(c == 1))
                        snap = act.tile([65, 256], bf16, tag="avown", bufs=16,
                                        name=f"oo{l}_{j}_{hh}")
                        nc.vector.tensor_copy(snap[:], avp[0:65, 0:256])
                        o_own[(j, hh)] = snap

                # Pass 2: stream the three OTHER ranks' K/V per j from the
                # gathered dram buffer; re-inject partial [o|Z]; head pairs
                # share PSUM banks -> one exp per chunk.
                for j in range(8):
                    k_j = act.tile([P, 768], bf16, tag="ksbj", bufs=3,
                                   name=f"kj{l}_{j}")
                    for i in range(3):
                        nc.sync.dma_start(
                            k_j[:, i * 256:(i + 1) * 256],
                            kv_out[bass.ds(oth_ranks[i], 1),
                                   j * 32768:(j + 1) * 32768].rearrange(
                                "o (p t) -> p (o t)", p=P))
                    v_j = act.tile([P, 6, 130], bf16, tag="vsbj", bufs=3,
                                   name=f"vj{l}_{j}")
                    for i in range(3):
                        nc.sync.dma_start(
                            v_j[:, 2 * i:2 * i + 2, :],
                            kv_out[bass.ds(oth_ranks[i], 1), KSZ:KVTOT]
                            .rearrange("o (th p f) -> p (o th) f", p=P,
                                       f=16 * 65)[:, :, 130 * j:130 * (j + 1)])
                    v_j_h = v_j.rearrange("p c (h g) -> p c h g", h=2, g=65)

                    av = ps.tile([P, 512], f32, tag="av", bufs=2,
                                 name=f"av{l}_{j}")
                    for hh in range(2):
                        # one start=True per bank tenancy: the 2nd region's
                        # has_written bits are already cleared by the 1st
                        nc.tensor.matmul(av[0:65, hh * 256:(hh + 1) * 256],
                                         lhsT=ident_bf[0:65, 0:65],
                                         rhs=o_own[(j, hh)][:],
                                         start=(hh == 0), stop=False)
                    # chunk-PAIRS share a PSUM bank (same lhsT row group ->
                    # serialized PE drains; packing the two head-halves
                    # instead would put different row groups on one bank =
                    # concurrent drains = PSUM collision)
                    for cp in range(3):
                        for hh in range(2):
                            base = hh * 64
                            sps = ps.tile([P, 512], f32, tag="att_s", bufs=4,
                                          name=f"sps{l}_{j}_{cp}_{hh}")
                            for ci in range(2):
                                c = 2 * cp + ci
                                nc.tensor.matmul(
                                    sps[:, ci * 256:(ci + 1) * 256],
                                    lhsT=k_j[base:base + 64, c * 128:(c + 1) * 128],
                                    rhs=q_all[base:base + 64, j, :],
                                    start=(ci == 0), stop=True)
                            e = act.tile([P, 512], bf16, tag="e", bufs=4,
                                         name=f"e{l}_{j}_{cp}_{hh}")
                            nc.scalar.activation(e[:], sps[:], AF.Exp, scale=scale)
                            for ci in range(2):
                                c = 2 * cp + ci
                                nc.tensor.matmul(
                                    av[0:65, hh * 256:(hh + 1) * 256],
                                    lhsT=v_j_h[:, c, hh, :],
                                    rhs=e[:, ci * 256:(ci + 1) * 256],
                                    start=False, stop=(c == 5))
                    recip = rows.tile([1, 512], f32, tag="row5", bufs=3)
                    nc.vector.reciprocal(recip[:], av[64:65, :])
                    bc_sb = act.tile([P, 512], f32, tag="bcsb", bufs=2)
                    nc.gpsimd.partition_broadcast(bc_sb[0:64, :], recip[:],
                                                  channels=64)
                    nc.vector.tensor_mul(o_sb[0:64, j, :], av[0:64, 0:256],
                                         bc_sb[0:64, 0:256])
                    o_st = act.tile([64, 256], bf16, tag="ost", bufs=2)
                    nc.vector.tensor_mul(o_st[:], av[0:64, 256:512],
                                         bc_sb[0:64, 256:512])
                    nc.sync.dma_start(o_sb[64:128, j, :], o_st[:])
                    # + v bias (sum of softmax weights == 1)
                    nc.vector.tensor_scalar_add(o_sb[:, j, :], o_sb[:, j, :],
                                                parms_t[:, 16 + j:16 + j + 1])

                # attention out-proj + residual
                for do in range(ND):
                    wb = wstr.tile([P, ND, P], bf16, tag="wb", bufs=6,
                                   name=f"wpj{l}_{do}")
                    nc.sync.dma_start(
                        wb[:].rearrange("p a b -> p (a b)"), pj_wb[l, do])
                    acc = ps.tile([P, 256], f32, tag="acc")
                    for dt in range(ND):
                        nc.tensor.matmul(acc[:], lhsT=wb[:, dt, :],
                                         rhs=o_sb[:, dt, :],
                                         start=(dt == 0), stop=(dt == ND - 1))
                    nc.vector.scalar_tensor_tensor(
                        out=x_sb[:, do, :], in0=acc[:],
                        scalar=parms_t[:, 24 + do:24 + do + 1],
                        in1=x_sb[:, do, :], op0=OP.add, op1=OP.add)

                # LN2 + MLP (fc1 -> gelu -> fc2 interleaved ft-outer;
                # fc2 accumulates into 4 [P,512] banks = 8 d-outs)
                h2_sb = act.tile([P, ND, 256], bf16, tag="h", bufs=2)
                _ln(nc, ps, act, rows, cons, x_sb, h2_sb)

                f2accs = [ps.tile([P, 512], f32, tag="att_s", bufs=4,
                                  name=f"f2acc{l}_{b}") for b in range(4)]
                for ft in range(NFT):
                    wb = wstr.tile([P, ND, P], bf16, tag="wb", bufs=6,
                                   name=f"wf1{l}_{ft}")
                    nc.sync.dma_start(
                        wb[:].rearrange("p a b -> p (a b)"), f1_wb[l, ft])
                    acc = ps.tile([P, 256], f32, tag="acc")
                    for dt in range(ND):
                        nc.tensor.matmul(acc[:], lhsT=wb[:, dt, :],
                                         rhs=h2_sb[:, dt, :],
                                         start=(dt == 0), stop=(dt == ND - 1))
                    h1 = act.tile([P, 256], bf16, tag="h1", bufs=6,
                                  name=f"h1{l}_{ft}")
                    nc.scalar.activation(h1[:], acc[:], AF.Gelu,
                                         bias=parms_t[:, 32 + ft:32 + ft + 1])
                    w2 = wstr.tile([P, D], bf16, tag="wf2", bufs=4,
                                   name=f"wf2{l}_{ft}")
                    nc.sync.dma_start(w2[:], f2_wb[l, ft])
                    for do in range(ND):
                        # start only on each bank's FIRST region (start=True
                        # clears has_written for the whole bank)
                        nc.tensor.matmul(
                            f2accs[do // 2][:, (do % 2) * 256:(do % 2 + 1) * 256],
                            lhsT=w2[:, do * P:(do + 1) * P], rhs=h1[:],
                            start=(ft == 0 and do % 2 == 0), stop=(ft == NFT - 1))
                for do in range(ND):
                    nc.vector.scalar_tensor_tensor(
                        out=x_sb[:, do, :],
                        in0=f2accs[do // 2][:, (do % 2) * 256:(do % 2 + 1) * 256],
                        scalar=parms_t[:, 64 + do:64 + do + 1],
                        in1=x_sb[:, do, :], op0=OP.add, op1=OP.add)

            # ---------------- final LN + AllGather + LM head ----------------
            xf_sb = act.tile([P, ND, 256], bf16, tag="h", bufs=2)
            _ln(nc, ps, act, rows, cons, x_sb, xf_sb)

            xf_in = dram.tile([ND, P, 256], bf16)
            xf_out = dram.tile([NCORES, ND, P, 256], bf16, addr_space="Shared")
            nc.sync.dma_start(xf_in.rearrange("d p t -> p d t"), xf_sb[:])
            if single:
                nc.sync.dma_start(xf_out[0], xf_in[:])
            else:
                nc.gpsimd.collective_compute(
                    "AllGather", OP.bypass, replica_groups=all_group,
                    ins=[xf_in.opt()], outs=[xf_out.opt()])

            # release layer-phase pools; LM phase gets all 8 PSUM banks
            act.release()
            ps.release()
            lmact = tc.alloc_tile_pool(name="lmact", bufs=1)
            psB = tc.alloc_tile_pool(name="psB", bufs=8, space="PSUM")

            xall = []
            for g in range(4):
                xt = lmact.tile([P, 2, NT], bf16, tag="xall", bufs=4,
                                name=f"xall{g}")
                for i in range(2):
                    dt = 2 * g + i
                    nc.sync.dma_start(
                        xt[:, i, :].rearrange("p (r t) -> p r t", r=NCORES),
                        xf_out[:, dt, :, :].rearrange("r p t -> p r t"))
                xall.append(xt)
            lmw = []
            for dt in range(ND):
                wt = wlm.tile([P, VSP], bf16, tag="w", name=f"lmw{dt}")
                nc.sync.dma_start(wt[:], lm_wT[dt * P:(dt + 1) * P, :])
                lmw.append(wt)
            lmb_row = rows.tile([1, VSP], f32, tag="lmbrow", bufs=1)
            nc.sync.dma_start(lmb_row[:], lm_b[None, :])
            lmb_bc = lmact.tile([P, VSP], f32, tag="lmbbc")
            nc.gpsimd.partition_broadcast(lmb_bc[:], lmb_row[:], channels=P)

            for tk in range(NT // P):
                accs = [psB.tile([P, 512], f32, tag="lmacc", name=f"lmacc{tk}_{v}")
                        for v in range(8)]
                for dt in range(ND):
                    lhs = xall[dt // 2][:, dt % 2, tk * P:(tk + 1) * P]
                    for vc in range(8):
                        nc.tensor.matmul(
                            accs[vc][:], lhsT=lhs,
                            rhs=lmw[dt][:, vc * 512:(vc + 1) * 512],
                            start=(dt == 0), stop=(dt == ND - 1))
                for vc in range(8):
                    osb = lmact.tile([P, 512], f32, tag="osb", bufs=4)
                    nc.vector.tensor_add(osb[:], accs[vc][:],
                                         lmb_bc[:, vc * 512:(vc + 1) * 512])
                    nc.sync.dma_start(
                        out_tok[tk * P:(tk + 1) * P, vc * 512:(vc + 1) * 512],
                        osb[:])
            lmact.release()
            psB.release()

    nc.compile()
    return nc


def _prep_in_maps(inputs, n_layers=L):
    input_ids = np.asarray(inputs["input_ids"]).reshape(NT).astype(np.int32)
    pos_w = np.asarray(inputs["pos_w"], dtype=np.float32)
    embed_w = np.ascontiguousarray(np.asarray(inputs["embed_w"], dtype=np.float32))

    f = np.float32
    attn_in_w = np.asarray(inputs["attn_in_w"], f)    # [L, 3D, D]
    attn_in_b = np.asarray(inputs["attn_in_b"], f)    # [L, 3D]
    ln1_s = np.asarray(inputs["ln1_s"], f)
    ln1_b = np.asarray(inputs["ln1_b"], f)
    fc1_w = np.asarray(inputs["fc1_w"], f)            # [L, F, D]
    fc1_b = np.asarray(inputs["fc1_b"], f)
    ln2_s = np.asarray(inputs["ln2_s"], f)
    ln2_b = np.asarray(inputs["ln2_b"], f)
    fc2_w = np.asarray(inputs["fc2_w"], f)            # [L, D, F]
    fc2_b = np.asarray(inputs["fc2_b"], f)
    proj_w = np.asarray(inputs["attn_out_w"], f)      # [L, D, D]
    proj_b = np.asarray(inputs["attn_out_b"], f)
    lm_w = np.asarray(inputs["lm_w"], f)
    lm_b_full = np.asarray(inputs["lm_b"], f)
    lnf_s = np.asarray(inputs["lnf_s"], f)
    lnf_b = np.asarray(inputs["lnf_b"], f)

    # fold LN scale into the next matmul's weights, LN bias into its bias
    attn_w_f = attn_in_w * ln1_s[:, None, :]          # [L, 3D, D]
    attn_b_f = attn_in_b + np.einsum("led,ld->le", attn_in_w, ln1_b)
    fc1_w_f = fc1_w * ln2_s[:, None, :]
    fc1_b_f = fc1_b + np.einsum("lfd,ld->lf", fc1_w, ln2_b)
    lm_w_f = lm_w * lnf_s[None, :]
    lm_b_f = lm_b_full + lm_w @ lnf_b

    bf = ml_dtypes.bfloat16
    # All lhsT bundles carry the CONTRACTION dim on partitions:
    # bundle[l, ftile, p, dt*128 + c] = W^T[dt*128 + p, ftile*128 + c].
    awT = np.transpose(attn_w_f, (0, 2, 1))           # [L, D(in), 3D(out)]
    qk_full = awT.reshape(L, ND, P, 24, P).transpose(0, 3, 2, 1, 4).reshape(
        L, 24, P, D)                                  # [L, ftile, p_in, dt*c]
    qk_order = np.concatenate([np.arange(8, 16), np.arange(0, 8)])  # k then q
    qk_wb = np.ascontiguousarray(qk_full[:, qk_order]).astype(bf)
    # v weights (moving operand): [L, 2nb, 8dt, 128 d_in_row, 512 vfeat]
    vwT = awT[:, :, 2 * D:3 * D]                      # [L, D(in), 1024 vf]
    v_wb = np.ascontiguousarray(
        vwT.reshape(L, ND, P, 2, 512).transpose(0, 3, 1, 2, 4)).astype(bf)
    # proj bundles: [L, 8do, 128 p_in, 8dt*128 out]
    pjT = np.transpose(proj_w, (0, 2, 1))             # [L, D(in), D(out)]
    pj_wb = np.ascontiguousarray(
        pjT.reshape(L, ND, P, ND, P).transpose(0, 3, 2, 1, 4).reshape(
            L, ND, P, D)).astype(bf)
    # fc1 bundles: [L, 32ft, 128 p_in, 8dt*128]
    f1T = np.transpose(fc1_w_f, (0, 2, 1))            # [L, D(in), F(out)]
    f1_wb = np.ascontiguousarray(
        f1T.reshape(L, ND, P, NFT, P).transpose(0, 3, 2, 1, 4).reshape(
            L, NFT, P, D)).astype(bf)
    # fc2 per-ft: [L, 32ft(in rows), 128 f_in_row, 1024 d_out]
    f2_wb = np.ascontiguousarray(
        np.transpose(fc2_w, (0, 2, 1)).reshape(L, NFT, P, D)).astype(bf)

    parms = np.zeros((L, P, NPAR), f)
    parms[:, :, 0:24] = attn_b_f.reshape(L, 24, P).transpose(0, 2, 1)
    parms[:, :, 24:32] = proj_b.reshape(L, 8, P).transpose(0, 2, 1)
    parms[:, :, 32:64] = fc1_b_f.reshape(L, 32, P).transpose(0, 2, 1)
    parms[:, :, 64:72] = fc2_b.reshape(L, 8, P).transpose(0, 2, 1)

    common = {
        "embed_w": embed_w,
        "qk_wb": qk_wb,
        "v_wb": v_wb,
        "pj_wb": pj_wb,
        "f1_wb": f1_wb,
        "f2_wb": f2_wb,
        "parms": parms,
    }

    in_maps = []
    for c in range(NCORES):
        s0 = (c % 4) * T
        lm_shard = np.zeros((VSP, D), f)
        lm_shard[:VS] = lm_w_f[c * VS:(c + 1) * VS]
        lmb_shard = np.zeros(VSP, f)
        lmb_shard[:VS] = lm_b_f[c * VS:(c + 1) * VS]
        m = dict(common)
        m["ids"] = input_ids[c * T:(c + 1) * T]
        m["pos"] = np.ascontiguousarray(pos_w[s0:s0 + T])
        m["lm_wT"] = np.ascontiguousarray(lm_shard.T).astype(bf)
        m["lm_b"] = lmb_shard
        in_maps.append(m)
    return in_maps


def _assemble(results):
    parts = [results[c]["out_tok"][:, :VS] for c in range(NCORES)]
    logits = np.concatenate(parts, axis=1)     # [2048, 32000]
    return np.ascontiguousarray(logits.reshape(B, S, V).astype(np.float32))


_NC_CACHE = {}


def _get_nc(n_layers=L):
    if n_layers not in _NC_CACHE:
        _NC_CACHE[n_layers] = build(n_layers)
    return _NC_CACHE[n_layers]


def run(inputs, n_layers=L, trace=False, trace_cores=None):
    nc = _get_nc(n_layers)
    in_maps = _prep_in_maps(inputs, n_layers)
    if trace:
        try:
            import axon_ntff_shim
            axon_ntff_shim.install()
        except Exception:
            pass
    res = bass_utils.run_bass_kernel_spmd(
        nc, in_maps, core_ids=list(range(NCORES)), trace=trace,
        trace_cores=(trace_cores or [0]) if trace else None)
    return _assemble(res.results), res


def kernel(**inputs) -> np.ndarray:
    out, _ = run(inputs)
    return out


# revision 3
# speedup vs baseline: 1.0905x; 1.0905x over previous
"""Trainium2 Bass kernel for a 4-layer transformer (B=2,S=1024,D=1024,H=16,F=4096,V=32000).

Strategy (8 NeuronCores): sequence-parallel layers (256 tokens/core, weights
replicated, streamed bf16), feature-major activations, per-layer merged K+V
AllGather within each batch's 4-core group, vocab-sharded LM head.

v2 changes vs baseline:
 - Weights streamed as per-output-tile bundles ([128, 8dt, 128] host-packed)
   through small rotating SBUF slots instead of 8 resident 1MB tiles; fc2 is
   consumed ft-outer right behind fc1+gelu, accumulating into 8 PSUM banks.
 - LN scale/bias folded into the following weights on host; device LN is pure
   standardize with DVE Newton-rsqrt (ACT only runs Exp + Gelu).
 - All small per-layer params packed into one [128, 72] tensor (one DMA).
 - K and V gathered in ONE AllGather per layer; gathered K/V streamed per-j.
 - Attention scores packed 2-per-PSUM-bank -> exp on [128,512] tiles.
 - PSUM evacuations/bias-adds moved from ACT to DVE.
"""
import numpy as np
import ml_dtypes

import concourse.bass as bass
import concourse.bacc as bacc
import concourse.mybir as mybir
import concourse.tile as tile
from concourse import bass_utils
from concourse.masks import make_identity

B, S, D, H, L, F, V = 2, 1024, 1024, 16, 4, 4096, 32000
DH = D // H          # 64
NCORES = 8
T = (B * S) // NCORES  # 256 tokens per core
NT = B * S             # 2048
VS = V // NCORES       # 4000
VSP = 4096             # padded vocab shard
P = 128
ND = D // P            # 8 d-tiles
NFT = F // P           # 32 fc1 f-tiles
NPAR = 72              # packed params: qkvb[0:24] projb[24:32] fc1b[32:64] fc2b[64:72]
KSZ = 8 * P * 256          # k elems in merged AG buffer
VSZ = 256 * 16 * 65        # v elems ([2 th, 128 p, 16h*65])
KVTOT = KSZ + VSZ

f32 = mybir.dt.float32
bf16 = mybir.dt.bfloat16
f8 = mybir.dt.float8e4
i32 = mybir.dt.int32
u32 = mybir.dt.uint32
AF = mybir.ActivationFunctionType
OP = mybir.AluOpType

RSQRT_MAGIC_P1 = 0x5F3759E0  # 0x5f3759df + 1 (two's-complement sub via xor+add)


def _ln(nc, ps, act, rows, cons, x_sb, out_h):
    """Standardize: x_sb [128, 8, 256] f32 -> out_h [128, 8, 256] bf16.

    (x - mu) * rsqrt(var + eps); LN scale/bias are folded into the next
    weight matrix on the host. Stats via paired N=512 ones-matmuls; rsqrt
    via DVE bit-hack seed + 2 Newton steps (ACT tables stay on exp/gelu)."""
    stat_x = ps.tile([1, 512], f32, tag="att_s", bufs=4)
    stat_q = ps.tile([1, 512], f32, tag="att_s", bufs=4)
    for dp in range(4):
        xp = x_sb[:, 2 * dp:2 * dp + 2, :].rearrange("p a b -> p (a b)")
        sq = act.tile([P, 512], f32, tag="sq", bufs=2)
        nc.vector.tensor_mul(sq[:], xp, xp)
        nc.tensor.matmul(stat_x[:], lhsT=cons.ones_col_f32[:], rhs=xp,
                         start=(dp == 0), stop=(dp == 3))
        nc.tensor.matmul(stat_q[:], lhsT=cons.ones_col_f32[:], rhs=sq[:],
                         start=(dp == 0), stop=(dp == 3))
    murow = rows.tile([1, 512], f32, tag="row")  # [mu | invstd]
    sxrow = rows.tile([1, 512], f32, tag="row")
    nc.vector.tensor_copy(sxrow[:], stat_x[:])
    sqrow = rows.tile([1, 512], f32, tag="row")
    nc.vector.tensor_copy(sqrow[:], stat_q[:])
    sumx = rows.tile([1, 256], f32, tag="row")
    nc.vector.tensor_add(sumx[:], sxrow[:, 0:256], sxrow[:, 256:512])
    nc.vector.tensor_scalar(murow[:, 0:256], sumx[:], 1.0 / D, None, OP.mult)
    msq = rows.tile([1, 256], f32, tag="row")
    nc.vector.tensor_add(msq[:], sqrow[:, 0:256], sqrow[:, 256:512])
    nc.vector.tensor_scalar(msq[:], msq[:], 1.0 / D, 1e-5, OP.mult, OP.add)
    mu2 = rows.tile([1, 256], f32, tag="row")
    nc.vector.tensor_mul(mu2[:], murow[:, 0:256], murow[:, 0:256])
    vare = rows.tile([1, 256], f32, tag="row")
    nc.vector.tensor_sub(vare[:], msq[:], mu2[:])
    std = rows.tile([1, 256], f32, tag="row")
    nc.scalar.activation(std[:], vare[:], AF.Sqrt)
    nc.vector.reciprocal(murow[:, 256:512], std[:])
    bc = act.tile([P, 512], f32, tag="lnbc", bufs=2)
    nc.gpsimd.partition_broadcast(bc[:], murow[:], channels=P)
    for dt in range(ND):
        t = act.tile([P, 256], f32, tag="lnt", bufs=3)
        nc.vector.tensor_sub(t[:], x_sb[:, dt, :], bc[:, 0:256])
        nc.vector.tensor_mul(out_h[:, dt, :], t[:], bc[:, 256:512])


class _Cons:
    pass


def build(n_layers=L, single=False):
    """single=True: 1-core variant with collectives replaced by local DMA
    copies (for TimelineSim cost-model analysis only — wrong numerics)."""
    nc = bacc.Bacc("TRN2", target_bir_lowering=False, debug=False,
                   num_devices=1 if single else NCORES)

    ids = nc.dram_tensor("ids", [T], i32, kind="ExternalInput").ap()
    pos = nc.dram_tensor("pos", [T, D], f32, kind="ExternalInput").ap()
    embed_w = nc.dram_tensor("embed_w", [V, D], f32, kind="ExternalInput").ap()
    # qk bundles: [L, 16, 128, 8dt*128] — k fts first (8), then q fts (8)
    qk_wb = nc.dram_tensor("qk_wb", [L, 16, P, ND * P], bf16, kind="ExternalInput").ap()
    # v weight stream: [L, 2nb, 8dt, 128, 512]
    v_wb = nc.dram_tensor("v_wb", [L, 2, ND, P, 512], bf16, kind="ExternalInput").ap()
    # proj bundles: [L, 8do, 128, 8dt*128]
    pj_wb = nc.dram_tensor("pj_wb", [L, ND, P, ND * P], bf16, kind="ExternalInput").ap()
    # fc1 bundles: [L, 32ft, 128, 8dt*128]
    f1_wb = nc.dram_tensor("f1_wb", [L, NFT, P, ND * P], bf16, kind="ExternalInput").ap()
    # fc2 per-ft rows: [L, 32ft, 128, 1024]
    f2_wb = nc.dram_tensor("f2_wb", [L, NFT, P, D], bf16, kind="ExternalInput").ap()
    parms = nc.dram_tensor("parms", [L, P, NPAR], f32, kind="ExternalInput").ap()
    lm_wT = nc.dram_tensor("lm_wT", [D, VSP], bf16, kind="ExternalInput").ap()
    lm_b = nc.dram_tensor("lm_b", [VSP], f32, kind="ExternalInput").ap()
    out_tok = nc.dram_tensor("out_tok", [NT, VSP], f32, kind="ExternalOutput").ap()

    kv_groups = [[0, 1, 2, 3], [4, 5, 6, 7]]
    all_group = [list(range(NCORES))]

    with tile.TileContext(nc) as tc:
        with (
            tc.tile_pool(name="consp", bufs=1) as consp,
            tc.tile_pool(name="wstr", bufs=1) as wstr,
            tc.tile_pool(name="rows", bufs=6) as rows,
            tc.tile_pool(name="par", bufs=2) as par,
            tc.tile_pool(name="dram", bufs=1, space="DRAM") as dram,
        ):
            act = tc.alloc_tile_pool(name="act", bufs=1)
            ps = tc.alloc_tile_pool(name="ps", bufs=2, space="PSUM")
            cons = _Cons()
            ident = consp.tile([P, P], f32)
            make_identity(nc, ident)
            ident_bf = consp.tile([P, P], bf16)
            nc.vector.tensor_copy(ident_bf[:], ident[:])
            ones_col_f32 = consp.tile([P, 1], f32)
            nc.vector.memset(ones_col_f32[:], 1.0)
            cons.ones_col_f32 = ones_col_f32

            x_sb = consp.tile([P, ND, 256], f32)  # residual, feature-major

            # per-core group-rank registers for own-block-skipping dynamic DMAs
            seng = nc.sync
            pid = seng.partition_id()
            rgrp = seng.alloc_register("grp_rank")
            seng.reg_alu(rgrp, pid, 3, OP.bitwise_and)
            grp_rank = seng.snap(rgrp, donate=True, min_val=0, max_val=3)
            oth_ranks = []
            for i in range(3):
                ra = seng.alloc_register(f"oth{i}a")
                seng.reg_alu(ra, grp_rank, i + 1, OP.add)
                rb = seng.alloc_register(f"oth{i}b")
                seng.reg_alu(rb, ra, 3, OP.bitwise_and)
                oth_ranks.append(seng.snap(rb, donate=True, min_val=0, max_val=3))

            # ---------------- embedding ----------------
            for tc2 in range(2):
                ids_sb = par.tile([P, 1], i32, tag="ids")
                nc.sync.dma_start(ids_sb[:], ids[tc2 * P:(tc2 + 1) * P, None])
                gat = act.tile([P, D], f32, tag="emb", bufs=2, name=f"gat{tc2}")
                nc.gpsimd.indirect_dma_start(
                    out=gat[:], out_offset=None, in_=embed_w[:],
                    in_offset=bass.IndirectOffsetOnAxis(ap=ids_sb[:, :1], axis=0))
                pos_sb = act.tile([P, D], f32, tag="emb", bufs=2, name=f"pos{tc2}")
                nc.sync.dma_start(pos_sb[:], pos[tc2 * P:(tc2 + 1) * P, :])
                nc.vector.tensor_add(gat[:], gat[:], pos_sb[:])
                for dt in range(ND):
                    tp = ps.tile([P, P], f32, tag="att_s", bufs=4)
                    nc.tensor.transpose(tp[:], gat[:, dt * P:(dt + 1) * P], ident[:])
                    nc.vector.tensor_copy(x_sb[:, dt, tc2 * P:(tc2 + 1) * P], tp[:])

            # ---------------- layers ----------------
            for l in range(n_layers):
                parms_t = par.tile([P, NPAR], f32, tag="lnp")
                nc.sync.dma_start(parms_t[:], parms[l])

                h_sb = act.tile([P, ND, 256], bf16, tag="h", bufs=2)
                _ln(nc, ps, act, rows, cons, x_sb, h_sb)

                q_all = act.tile([P, 8, 256], bf16, tag="q")
                k_loc = act.tile([P, 8, 256], bf16, tag="kloc")
                # K first so the AllGather can start as early as possible
                for kf in range(8):
                    wb = wstr.tile([P, ND, P], bf16, tag="wb", bufs=6,
                                   name=f"wk{l}_{kf}")
                    nc.sync.dma_start(
                        wb[:].rearrange("p a b -> p (a b)"), qk_wb[l, kf])
                    acc = ps.tile([P, 256], f32, tag="acc")
                    for dt in range(ND):
                        nc.tensor.matmul(acc[:], lhsT=wb[:, dt, :],
                                         rhs=h_sb[:, dt, :],
                                         start=(dt == 0), stop=(dt == ND - 1))
                    nc.vector.tensor_scalar_add(k_loc[:, kf, :], acc[:],
                                                parms_t[:, 8 + kf:8 + kf + 1])

                # V (token-major, per head 65 cols = [v_h | 1])
                v_loc = act.tile([P, 2, 16 * 65], bf16, tag="vloc")
                v_loc_h = v_loc.rearrange("p c (h g) -> p c h g", h=16, g=65)
                for nb in range(2):
                    accs = [ps.tile([P, 512], f32, tag="av", bufs=2,
                                    name=f"vacc{l}_{nb}_{tc2}")
                            for tc2 in range(2)]
                    for dt in range(ND):
                        wv = wstr.tile([P, 512], bf16, tag="wv", bufs=4,
                                       name=f"wv{l}_{nb}_{dt}")
                        nc.sync.dma_start(wv[:], v_wb[l, nb, dt])
                        for tc2 in range(2):
                            nc.tensor.matmul(
                                accs[tc2][:],
                                lhsT=h_sb[:, dt, tc2 * P:(tc2 + 1) * P],
                                rhs=wv[:],
                                start=(dt == 0), stop=(dt == ND - 1))
                    for tc2 in range(2):
                        nc.vector.tensor_copy(
                            v_loc_h[:, tc2, nb * 8:(nb + 1) * 8, 0:64],
                            accs[tc2][:].rearrange("p (h g) -> p h g", h=8))
                for tc2 in range(2):
                    nc.vector.memset(v_loc_h[:, tc2, :, 64:65], 1.0)

                # merged K+V bounce + single AllGather per layer
                kv_in = dram.tile([KVTOT], bf16, tag="kvin", name=f"kvin{l}")
                kv_out = dram.tile([4, KVTOT], bf16, tag="kvout", name=f"kvout{l}")
                nc.sync.dma_start(
                    kv_in[0:KSZ].rearrange("(f p t) -> p f t", f=8, p=P), k_loc[:])
                for tc2 in range(2):
                    nc.sync.dma_start(
                        kv_in[KSZ + tc2 * P * 1040:KSZ + (tc2 + 1) * P * 1040]
                        .rearrange("(p f) -> p f", p=P),
                        v_loc[:, tc2, :])
                if single:
                    nc.sync.dma_start(kv_out[0], kv_in[:])
                else:
                    nc.gpsimd.collective_compute(
                        "AllGather", OP.bypass, replica_groups=kv_groups,
                        ins=[kv_in.opt()], outs=[kv_out.opt()])

                # Q projections (overlap the AllGather)
                for qf in range(8):
                    wb = wstr.tile([P, ND, P], bf16, tag="wb", bufs=6,
                                   name=f"wq{l}_{qf}")
                    nc.sync.dma_start(
                        wb[:].rearrange("p a b -> p (a b)"), qk_wb[l, 8 + qf])
                    acc = ps.tile([P, 256], f32, tag="acc")
                    for dt in range(ND):
                        nc.tensor.matmul(acc[:], lhsT=wb[:, dt, :],
                                         rhs=h_sb[:, dt, :],
                                         start=(dt == 0), stop=(dt == ND - 1))
                    nc.vector.tensor_scalar_add(q_all[:, qf, :], acc[:],
                                                parms_t[:, qf:qf + 1])

                o_sb = act.tile([P, ND, 256], bf16, tag="o")
                scale = 1.0 / np.sqrt(DH)

                # Pass 1 (pre-AllGather): attention over this core's OWN 256
                # k-tokens; both k-chunks share one PSUM bank -> one exp each.
                o_own = {}
                for j in range(8):
                    for hh in range(2):
                        h_idx = 2 * j + hh
                        base = hh * 64
                        sps = ps.tile([P, 512], f32, tag="att_s", bufs=4,
                                      name=f"spp{l}_{j}_{hh}")
                        for c in range(2):
                            nc.tensor.matmul(
                                sps[:, c * 256:(c + 1) * 256],
                                lhsT=k_loc[base:base + 64, j, c * P:(c + 1) * P],
                                rhs=q_all[base:base + 64, j, :],
                                start=(c == 0), stop=True)
                        e = act.tile([P, 512], bf16, tag="e", bufs=4,
                                     name=f"ep{l}_{j}_{hh}")
                        nc.scalar.activation(e[:], sps[:], AF.Exp, scale=scale)
                        avp = ps.tile([P, 512], f32, tag="av", bufs=2,
                                      name=f"avp{l}_{j}_{hh}")
                        for c in range(2):
                            nc.tensor.matmul(
                                avp[0:65, 0:256],
                                lhsT=v_loc_h[:, c, h_idx, :],
                                rhs=e[:, c * 256:(c + 1) * 256],
                                start=(c == 0), stop=(c == 1))
                        snap = act.tile([65, 256], bf16, tag="avown", bufs=16,
                                        name=f"oo{l}_{j}_{hh}")
                        nc.vector.tensor_copy(snap[:], avp[0:65, 0:256])
                        o_own[(j, hh)] = snap

                # Pass 2: stream the three OTHER ranks' K/V per j from the
                # gathered dram buffer; re-inject partial [o|Z]; head pairs
                # share PSUM banks -> one exp per chunk.
                k_all = act.tile([P, 8, 768], bf16, tag="ksba",
                                 name=f"kall{l}")
                for i in range(3):
                    nc.sync.dma_start(
                        k_all[:, :, i * 256:(i + 1) * 256],
                        kv_out[bass.ds(oth_ranks[i], 1), 0:KSZ].rearrange(
                            "o (f p t) -> p f (o t)", f=8, p=P))
                v_all = act.tile([P, 6, 16 * 65], bf16, tag="vsba",
                                 name=f"vall{l}")
                for i in range(3):
                    nc.sync.dma_start(
                        v_all[:, 2 * i:2 * i + 2, :],
                        kv_out[bass.ds(oth_ranks[i], 1), KSZ:KVTOT]
                        .rearrange("o (th p f) -> p (o th) f", p=P,
                                   f=16 * 65))
                v_all_h = v_all.rearrange("p c (h g) -> p c h g", h=16, g=65)
                for j in range(8):
                    k_j = k_all[:, j, :]

                    av = ps.tile([P, 512], f32, tag="av", bufs=2,
                                 name=f"av{l}_{j}")
                    for hh in range(2):
                        # one start=True per bank tenancy: the 2nd region's
                        # has_written bits are already cleared by the 1st
                        nc.tensor.matmul(av[0:65, hh * 256:(hh + 1) * 256],
                                         lhsT=ident_bf[0:65, 0:65],
                                         rhs=o_own[(j, hh)][:],
                                         start=(hh == 0), stop=False)
                    # chunk-PAIRS share a PSUM bank (same lhsT row group ->
                    # serialized PE drains; packing the two head-halves
                    # instead would put different row groups on one bank =
                    # concurrent drains = PSUM collision)
                    for cp in range(3):
                        for hh in range(2):
                            base = hh * 64
                            sps = ps.tile([P, 512], f32, tag="att_s", bufs=4,
                                          name=f"sps{l}_{j}_{cp}_{hh}")
                            for ci in range(2):
                                c = 2 * cp + ci
                                nc.tensor.matmul(
                                    sps[:, ci * 256:(ci + 1) * 256],
                                    lhsT=k_j[base:base + 64, c * 128:(c + 1) * 128],
                                    rhs=q_all[base:base + 64, j, :],
                                    start=(ci == 0), stop=True)
                            e = act.tile([P, 512], bf16, tag="e", bufs=4,
                                         name=f"e{l}_{j}_{cp}_{hh}")
                            nc.scalar.activation(e[:], sps[:], AF.Exp, scale=scale)
                            for ci in range(2):
                                c = 2 * cp + ci
                                nc.tensor.matmul(
                                    av[0:65, hh * 256:(hh + 1) * 256],
                                    lhsT=v_all_h[:, c, 2 * j + hh, :],
                                    rhs=e[:, ci * 256:(ci + 1) * 256],
                                    start=False, stop=(c == 5))
                    recip = rows.tile([1, 512], f32, tag="row5", bufs=3)
                    nc.vector.reciprocal(recip[:], av[64:65, :])
                    bc_sb = act.tile([P, 512], f32, tag="bcsb", bufs=2)
                    nc.gpsimd.partition_broadcast(bc_sb[0:64, :], recip[:],
                                                  channels=64)
                    nc.vector.tensor_mul(o_sb[0:64, j, :], av[0:64, 0:256],
                                         bc_sb[0:64, 0:256])
                    o_st = act.tile([64, 256], bf16, tag="ost", bufs=2)
                    nc.vector.tensor_mul(o_st[:], av[0:64, 256:512],
                                         bc_sb[0:64, 256:512])
                    nc.sync.dma_start(o_sb[64:128, j, :], o_st[:])
                    # + v bias (sum of softmax weights == 1)
                    nc.vector.tensor_scalar_add(o_sb[:, j, :], o_sb[:, j, :],
                                                parms_t[:, 16 + j:16 + j + 1])

                # attention out-proj + residual
                for do in range(ND):
                    wb = wstr.tile([P, ND, P], bf16, tag="wb", bufs=6,
                                   name=f"wpj{l}_{do}")
                    nc.sync.dma_start(
                        wb[:].rearrange("p a b -> p (a b)"), pj_wb[l, do])
                    acc = ps.tile([P, 256], f32, tag="acc")
                    for dt in range(ND):
                        nc.tensor.matmul(acc[:], lhsT=wb[:, dt, :],
                                         rhs=o_sb[:, dt, :],
                                         start=(dt == 0), stop=(dt == ND - 1))
                    nc.vector.scalar_tensor_tensor(
                        out=x_sb[:, do, :], in0=acc[:],
                        scalar=parms_t[:, 24 + do:24 + do + 1],
                        in1=x_sb[:, do, :], op0=OP.add, op1=OP.add)

                # LN2 + MLP (fc1 -> gelu -> fc2 interleaved ft-outer;
                # fc2 accumulates into 4 [P,512] banks = 8 d-outs)
                h2_sb = act.tile([P, ND, 256], bf16, tag="h", bufs=2)
                _ln(nc, ps, act, rows, cons, x_sb, h2_sb)

                f2accs = [ps.tile([P, 512], f32, tag="att_s", bufs=4,
                                  name=f"f2acc{l}_{b}") for b in range(4)]
                for ft in range(NFT):
                    wb = wstr.tile([P, ND, P], bf16, tag="wb", bufs=6,
                                   name=f"wf1{l}_{ft}")
                    nc.sync.dma_start(
                        wb[:].rearrange("p a b -> p (a b)"), f1_wb[l, ft])
                    acc = ps.tile([P, 256], f32, tag="acc")
                    for dt in range(ND):
                        nc.tensor.matmul(acc[:], lhsT=wb[:, dt, :],
                                         rhs=h2_sb[:, dt, :],
                                         start=(dt == 0), stop=(dt == ND - 1))
                    h1 = act.tile([P, 256], bf16, tag="h1", bufs=6,
                                  name=f"h1{l}_{ft}")
                    nc.scalar.activation(h1[:], acc[:], AF.Gelu,
                                         bias=parms_t[:, 32 + ft:32 + ft + 1])
                    w2 = wstr.tile([P, D], bf16, tag="wf2", bufs=4,
                                   name=f"wf2{l}_{ft}")
                    nc.sync.dma_start(w2[:], f2_wb[l, ft])
                    for do in range(ND):
                        # start only on each bank's FIRST region (start=True
                        # clears has_written for the whole bank)
                        nc.tensor.matmul(
                            f2accs[do // 2][:, (do % 2) * 256:(do % 2 + 1) * 256],
                            lhsT=w2[:, do * P:(do + 1) * P], rhs=h1[:],
                            start=(ft == 0 and do % 2 == 0), stop=(ft == NFT - 1))
                for do in range(ND):
                    nc.vector.scalar_tensor_tensor(
                        out=x_sb[:, do, :],
                        in0=f2accs[do // 2][:, (do % 2) * 256:(do % 2 + 1) * 256],
                        scalar=parms_t[:, 64 + do:64 + do + 1],
                        in1=x_sb[:, do, :], op0=OP.add, op1=OP.add)

            # ---------------- final LN + AllGather + LM head ----------------
            xf_sb = act.tile([P, ND, 256], bf16, tag="h", bufs=2)
            _ln(nc, ps, act, rows, cons, x_sb, xf_sb)

            xf_in = dram.tile([ND, P, 256], bf16)
            xf_out = dram.tile([NCORES, ND, P, 256], bf16, addr_space="Shared")
            nc.sync.dma_start(xf_in.rearrange("d p t -> p d t"), xf_sb[:])
            if single:
                nc.sync.dma_start(xf_out[0], xf_in[:])
            else:
                nc.gpsimd.collective_compute(
                    "AllGather", OP.bypass, replica_groups=all_group,
                    ins=[xf_in.opt()], outs=[xf_out.opt()])

            # release layer-phase pools; LM phase gets all 8 PSUM banks
            act.release()
            ps.release()
            lmact = tc.alloc_tile_pool(name="lmact", bufs=1)
            psB = tc.alloc_tile_pool(name="psB", bufs=8, space="PSUM")

            xall = []
            for g in range(4):
                xt = lmact.tile([P, 2, NT], bf16, tag="xall", bufs=4,
                                name=f"xall{g}")
                for i in range(2):
                    dt = 2 * g + i
                    nc.sync.dma_start(
                        xt[:, i, :].rearrange("p (r t) -> p r t", r=NCORES),
                        xf_out[:, dt, :, :].rearrange("r p t -> p r t"))
                xall.append(xt)
            lmw = []
            for dt in range(ND):
                wt = lmact.tile([P, VSP], bf16, tag="lmw", bufs=8,
                                name=f"lmw{dt}")
                nc.sync.dma_start(wt[:], lm_wT[dt * P:(dt + 1) * P, :])
                lmw.append(wt)
            lmb_row = rows.tile([1, VSP], f32, tag="lmbrow", bufs=1)
            nc.sync.dma_start(lmb_row[:], lm_b[None, :])
            lmb_bc = lmact.tile([P, VSP], f32, tag="lmbbc")
            nc.gpsimd.partition_broadcast(lmb_bc[:], lmb_row[:], channels=P)

            for tk in range(NT // P):
                accs = [psB.tile([P, 512], f32, tag="lmacc", name=f"lmacc{tk}_{v}")
                        for v in range(8)]
                for dt in range(ND):
                    lhs = xall[dt // 2][:, dt % 2, tk * P:(tk + 1) * P]
                    for vc in range(8):
                        nc.tensor.matmul(
                            accs[vc][:], lhsT=lhs,
                            rhs=lmw[dt][:, vc * 512:(vc + 1) * 512],
                            start=(dt == 0), stop=(dt == ND - 1))
                for vc in range(8):
                    osb = lmact.tile([P, 512], f32, tag="osb", bufs=4)
                    nc.vector.tensor_add(osb[:], accs[vc][:],
                                         lmb_bc[:, vc * 512:(vc + 1) * 512])
                    nc.sync.dma_start(
                        out_tok[tk * P:(tk + 1) * P, vc * 512:(vc + 1) * 512],
                        osb[:])
            lmact.release()
            psB.release()

    nc.compile()
    return nc


def _prep_in_maps(inputs, n_layers=L):
    input_ids = np.asarray(inputs["input_ids"]).reshape(NT).astype(np.int32)
    pos_w = np.asarray(inputs["pos_w"], dtype=np.float32)
    embed_w = np.ascontiguousarray(np.asarray(inputs["embed_w"], dtype=np.float32))

    f = np.float32
    attn_in_w = np.asarray(inputs["attn_in_w"], f)    # [L, 3D, D]
    attn_in_b = np.asarray(inputs["attn_in_b"], f)    # [L, 3D]
    ln1_s = np.asarray(inputs["ln1_s"], f)
    ln1_b = np.asarray(inputs["ln1_b"], f)
    fc1_w = np.asarray(inputs["fc1_w"], f)            # [L, F, D]
    fc1_b = np.asarray(inputs["fc1_b"], f)
    ln2_s = np.asarray(inputs["ln2_s"], f)
    ln2_b = np.asarray(inputs["ln2_b"], f)
    fc2_w = np.asarray(inputs["fc2_w"], f)            # [L, D, F]
    fc2_b = np.asarray(inputs["fc2_b"], f)
    proj_w = np.asarray(inputs["attn_out_w"], f)      # [L, D, D]
    proj_b = np.asarray(inputs["attn_out_b"], f)
    lm_w = np.asarray(inputs["lm_w"], f)
    lm_b_full = np.asarray(inputs["lm_b"], f)
    lnf_s = np.asarray(inputs["lnf_s"], f)
    lnf_b = np.asarray(inputs["lnf_b"], f)

    # fold LN scale into the next matmul's weights, LN bias into its bias
    attn_w_f = attn_in_w * ln1_s[:, None, :]          # [L, 3D, D]
    attn_b_f = attn_in_b + np.einsum("led,ld->le", attn_in_w, ln1_b)
    fc1_w_f = fc1_w * ln2_s[:, None, :]
    fc1_b_f = fc1_b + np.einsum("lfd,ld->lf", fc1_w, ln2_b)
    lm_w_f = lm_w * lnf_s[None, :]
    lm_b_f = lm_b_full + lm_w @ lnf_b

    bf = ml_dtypes.bfloat16
    # All lhsT bundles carry the CONTRACTION dim on partitions:
    # bundle[l, ftile, p, dt*128 + c] = W^T[dt*128 + p, ftile*128 + c].
    awT = np.transpose(attn_w_f, (0, 2, 1))           # [L, D(in), 3D(out)]
    qk_full = awT.reshape(L, ND, P, 24, P).transpose(0, 3, 2, 1, 4).reshape(
        L, 24, P, D)                                  # [L, ftile, p_in, dt*c]
    qk_order = np.concatenate([np.arange(8, 16), np.arange(0, 8)])  # k then q
    qk_wb = np.ascontiguousarray(qk_full[:, qk_order]).astype(bf)
    # v weights (moving operand): [L, 2nb, 8dt, 128 d_in_row, 512 vfeat]
    vwT = awT[:, :, 2 * D:3 * D]                      # [L, D(in), 1024 vf]
    v_wb = np.ascontiguousarray(
        vwT.reshape(L, ND, P, 2, 512).transpose(0, 3, 1, 2, 4)).astype(bf)
    # proj bundles: [L, 8do, 128 p_in, 8dt*128 out]
    pjT = np.transpose(proj_w, (0, 2, 1))             # [L, D(in), D(out)]
    pj_wb = np.ascontiguousarray(
        pjT.reshape(L, ND, P, ND, P).transpose(0, 3, 2, 1, 4).reshape(
            L, ND, P, D)).astype(bf)
    # fc1 bundles: [L, 32ft, 128 p_in, 8dt*128]
    f1T = np.transpose(fc1_w_f, (0, 2, 1))            # [L, D(in), F(out)]
    f1_wb = np.ascontiguousarray(
        f1T.reshape(L, ND, P, NFT, P).transpose(0, 3, 2, 1, 4).reshape(
            L, NFT, P, D)).astype(bf)
    # fc2 per-ft: [L, 32ft(in rows), 128 f_in_row, 1024 d_out]
    f2_wb = np.ascontiguousarray(
        np.transpose(fc2_w, (0, 2, 1)).reshape(L, NFT, P, D)).astype(bf)

    parms = np.zeros((L, P, NPAR), f)
    parms[:, :, 0:24] = attn_b_f.reshape(L, 24, P).transpose(0, 2, 1)
    parms[:, :, 24:32] = proj_b.reshape(L, 8, P).transpose(0, 2, 1)
    parms[:, :, 32:64] = fc1_b_f.reshape(L, 32, P).transpose(0, 2, 1)
    parms[:, :, 64:72] = fc2_b.reshape(L, 8, P).transpose(0, 2, 1)

    common = {
        "embed_w": embed_w,
        "qk_wb": qk_wb,
        "v_wb": v_wb,
        "pj_wb": pj_wb,
        "f1_wb": f1_wb,
        "f2_wb": f2_wb,
        "parms": parms,
    }

    in_maps = []
    for c in range(NCORES):
        s0 = (c % 4) * T
        lm_shard = np.zeros((VSP, D), f)
        lm_shard[:VS] = lm_w_f[c * VS:(c + 1) * VS]
        lmb_shard = np.zeros(VSP, f)
        lmb_shard[:VS] = lm_b_f[c * VS:(c + 1) * VS]
        m = dict(common)
        m["ids"] = input_ids[c * T:(c + 1) * T]
        m["pos"] = np.ascontiguousarray(pos_w[s0:s0 + T])
        m["lm_wT"] = np.ascontiguousarray(lm_shard.T).astype(bf)
        m["lm_b"] = lmb_shard
        in_maps.append(m)
    return in_maps


def _assemble(results):
    parts = [results[c]["out_tok"][:, :VS] for c in range(NCORES)]
    logits = np.concatenate(parts, axis=1)     # [2048, 32000]
    return np.ascontiguousarray(logits.reshape(B, S, V).astype(np.float32))


_NC_CACHE = {}


def _get_nc(n_layers=L):
    if n_layers not in _NC_CACHE:
        _NC_CACHE[n_layers] = build(n_layers)
    return _NC_CACHE[n_layers]


def run(inputs, n_layers=L, trace=False, trace_cores=None):
    nc = _get_nc(n_layers)
    in_maps = _prep_in_maps(inputs, n_layers)
    if trace:
        try:
            import axon_ntff_shim
            axon_ntff_shim.install()
        except Exception:
            pass
    res = bass_utils.run_bass_kernel_spmd(
        nc, in_maps, core_ids=list(range(NCORES)), trace=trace,
        trace_cores=(trace_cores or [0]) if trace else None)
    return _assemble(res.results), res


def kernel(**inputs) -> np.ndarray:
    out, _ = run(inputs)
    return out


# revision 4
# speedup vs baseline: 1.1088x; 1.0168x over previous
"""Trainium2 Bass kernel for a 4-layer transformer (B=2,S=1024,D=1024,H=16,F=4096,V=32000).

Strategy (8 NeuronCores): sequence-parallel layers (256 tokens/core, weights
replicated, streamed bf16), feature-major activations, per-layer merged K+V
AllGather within each batch's 4-core group, vocab-sharded LM head.

v2 changes vs baseline:
 - Weights streamed as per-output-tile bundles ([128, 8dt, 128] host-packed)
   through small rotating SBUF slots instead of 8 resident 1MB tiles; fc2 is
   consumed ft-outer right behind fc1+gelu, accumulating into 8 PSUM banks.
 - LN scale/bias folded into the following weights on host; device LN is pure
   standardize with DVE Newton-rsqrt (ACT only runs Exp + Gelu).
 - All small per-layer params packed into one [128, 72] tensor (one DMA).
 - K and V gathered in ONE AllGather per layer; gathered K/V streamed per-j.
 - Attention scores packed 2-per-PSUM-bank -> exp on [128,512] tiles.
 - PSUM evacuations/bias-adds moved from ACT to DVE.
"""
import numpy as np
import ml_dtypes

import concourse.bass as bass
import concourse.bacc as bacc
import concourse.mybir as mybir
import concourse.tile as tile
from concourse import bass_utils
from concourse.masks import make_identity

B, S, D, H, L, F, V = 2, 1024, 1024, 16, 4, 4096, 32000
DH = D // H          # 64
NCORES = 8
T = (B * S) // NCORES  # 256 tokens per core
NT = B * S             # 2048
VS = V // NCORES       # 4000
VSP = 4096             # padded vocab shard
P = 128
ND = D // P            # 8 d-tiles
NFT = F // P           # 32 fc1 f-tiles
NPAR = 72              # packed params: qkvb[0:24] projb[24:32] fc1b[32:64] fc2b[64:72]
KSZ = 8 * P * 256          # k elems in merged AG buffer
VSZ = 256 * 16 * 65        # v elems ([2 th, 128 p, 16h*65])
KVTOT = KSZ + VSZ
KH = 4 * P * 256           # k elems per half (4 j-tiles)
VH = 256 * 8 * 65          # v elems per half (8 heads)
KVH = KH + VH              # one half-AG payload

f32 = mybir.dt.float32
bf16 = mybir.dt.bfloat16
f8 = mybir.dt.float8e4
i32 = mybir.dt.int32
u32 = mybir.dt.uint32
AF = mybir.ActivationFunctionType
OP = mybir.AluOpType

RSQRT_MAGIC_P1 = 0x5F3759E0  # 0x5f3759df + 1 (two's-complement sub via xor+add)


def _ln(nc, ps, act, rows, cons, x_sb, out_h):
    """Standardize: x_sb [128, 8, 256] f32 -> out_h [128, 8, 256] bf16.

    (x - mu) * rsqrt(var + eps); LN scale/bias are folded into the next
    weight matrix on the host. Stats via paired N=512 ones-matmuls; rsqrt
    via DVE bit-hack seed + 2 Newton steps (ACT tables stay on exp/gelu)."""
    stat_x = ps.tile([1, 512], f32, tag="att_s", bufs=4)
    stat_q = ps.tile([1, 512], f32, tag="att_s", bufs=4)
    for dp in range(4):
        xp = x_sb[:, 2 * dp:2 * dp + 2, :].rearrange("p a b -> p (a b)")
        sq = act.tile([P, 512], f32, tag="sq", bufs=2)
        nc.vector.tensor_mul(sq[:], xp, xp)
        nc.tensor.matmul(stat_x[:], lhsT=cons.ones_col_f32[:], rhs=xp,
                         start=(dp == 0), stop=(dp == 3))
        nc.tensor.matmul(stat_q[:], lhsT=cons.ones_col_f32[:], rhs=sq[:],
                         start=(dp == 0), stop=(dp == 3))
    murow = rows.tile([1, 512], f32, tag="row")  # [mu | invstd]
    sxrow = rows.tile([1, 512], f32, tag="row")
    nc.vector.tensor_copy(sxrow[:], stat_x[:])
    sqrow = rows.tile([1, 512], f32, tag="row")
    nc.vector.tensor_copy(sqrow[:], stat_q[:])
    sumx = rows.tile([1, 256], f32, tag="row")
    nc.vector.tensor_add(sumx[:], sxrow[:, 0:256], sxrow[:, 256:512])
    nc.vector.tensor_scalar(murow[:, 0:256], sumx[:], 1.0 / D, None, OP.mult)
    msq = rows.tile([1, 256], f32, tag="row")
    nc.vector.tensor_add(msq[:], sqrow[:, 0:256], sqrow[:, 256:512])
    nc.vector.tensor_scalar(msq[:], msq[:], 1.0 / D, 1e-5, OP.mult, OP.add)
    mu2 = rows.tile([1, 256], f32, tag="row")
    nc.vector.tensor_mul(mu2[:], murow[:, 0:256], murow[:, 0:256])
    vare = rows.tile([1, 256], f32, tag="row")
    nc.vector.tensor_sub(vare[:], msq[:], mu2[:])
    std = rows.tile([1, 256], f32, tag="row")
    nc.scalar.activation(std[:], vare[:], AF.Sqrt)
    nc.vector.reciprocal(murow[:, 256:512], std[:])
    bc = act.tile([P, 512], f32, tag="lnbc", bufs=2)
    nc.gpsimd.partition_broadcast(bc[:], murow[:], channels=P)
    for dt in range(ND):
        t = act.tile([P, 256], f32, tag="lnt", bufs=3)
        nc.vector.tensor_sub(t[:], x_sb[:, dt, :], bc[:, 0:256])
        nc.vector.tensor_mul(out_h[:, dt, :], t[:], bc[:, 256:512])


class _Cons:
    pass


def build(n_layers=L, single=False):
    """single=True: 1-core variant with collectives replaced by local DMA
    copies (for TimelineSim cost-model analysis only — wrong numerics)."""
    nc = bacc.Bacc("TRN2", target_bir_lowering=False, debug=False,
                   num_devices=1 if single else NCORES)

    ids = nc.dram_tensor("ids", [T], i32, kind="ExternalInput").ap()
    pos = nc.dram_tensor("pos", [T, D], f32, kind="ExternalInput").ap()
    embed_w = nc.dram_tensor("embed_w", [V, D], f32, kind="ExternalInput").ap()
    # qk bundles: [L, 16, 128, 8dt*128] — k fts first (8), then q fts (8)
    qk_wb = nc.dram_tensor("qk_wb", [L, 16, P, ND * P], bf16, kind="ExternalInput").ap()
    # v weight stream: [L, 2nb, 8dt, 128, 512]
    v_wb = nc.dram_tensor("v_wb", [L, 2, ND, P, 512], bf16, kind="ExternalInput").ap()
    # proj bundles: [L, 8do, 128, 8dt*128]
    pj_wb = nc.dram_tensor("pj_wb", [L, ND, P, ND * P], bf16, kind="ExternalInput").ap()
    # fc1 bundles: [L, 32ft, 128, 8dt*128]
    f1_wb = nc.dram_tensor("f1_wb", [L, NFT, P, ND * P], bf16, kind="ExternalInput").ap()
    # fc2 per-ft rows: [L, 32ft, 128, 1024]
    f2_wb = nc.dram_tensor("f2_wb", [L, NFT, P, D], bf16, kind="ExternalInput").ap()
    parms = nc.dram_tensor("parms", [L, P, NPAR], f32, kind="ExternalInput").ap()
    lm_wT = nc.dram_tensor("lm_wT", [D, VSP], bf16, kind="ExternalInput").ap()
    lm_b = nc.dram_tensor("lm_b", [VSP], f32, kind="ExternalInput").ap()
    out_tok = nc.dram_tensor("out_tok", [NT, VSP], f32, kind="ExternalOutput").ap()

    kv_groups = [[0, 1, 2, 3], [4, 5, 6, 7]]
    all_group = [list(range(NCORES))]

    with tile.TileContext(nc) as tc:
        with (
            tc.tile_pool(name="consp", bufs=1) as consp,
            tc.tile_pool(name="wstr", bufs=1) as wstr,
            tc.tile_pool(name="rows", bufs=6) as rows,
            tc.tile_pool(name="par", bufs=2) as par,
            tc.tile_pool(name="dram", bufs=1, space="DRAM") as dram,
        ):
            act = tc.alloc_tile_pool(name="act", bufs=1)
            ps = tc.alloc_tile_pool(name="ps", bufs=2, space="PSUM")
            cons = _Cons()
            ident = consp.tile([P, P], f32)
            make_identity(nc, ident)
            ident_bf = consp.tile([P, P], bf16)
            nc.vector.tensor_copy(ident_bf[:], ident[:])
            ones_col_f32 = consp.tile([P, 1], f32)
            nc.vector.memset(ones_col_f32[:], 1.0)
            cons.ones_col_f32 = ones_col_f32

            x_sb = consp.tile([P, ND, 256], f32)  # residual, feature-major

            # per-core group-rank registers for own-block-skipping dynamic DMAs
            seng = nc.sync
            pid = seng.partition_id()
            rgrp = seng.alloc_register("grp_rank")
            seng.reg_alu(rgrp, pid, 3, OP.bitwise_and)
            grp_rank = seng.snap(rgrp, donate=True, min_val=0, max_val=3)
            oth_ranks = []
            for i in range(3):
                ra = seng.alloc_register(f"oth{i}a")
                seng.reg_alu(ra, grp_rank, i + 1, OP.add)
                rb = seng.alloc_register(f"oth{i}b")
                seng.reg_alu(rb, ra, 3, OP.bitwise_and)
                oth_ranks.append(seng.snap(rb, donate=True, min_val=0, max_val=3))

            # ---------------- embedding ----------------
            for tc2 in range(2):
                ids_sb = par.tile([P, 1], i32, tag="ids")
                nc.sync.dma_start(ids_sb[:], ids[tc2 * P:(tc2 + 1) * P, None])
                gat = act.tile([P, D], f32, tag="emb", bufs=2, name=f"gat{tc2}")
                nc.gpsimd.indirect_dma_start(
                    out=gat[:], out_offset=None, in_=embed_w[:],
                    in_offset=bass.IndirectOffsetOnAxis(ap=ids_sb[:, :1], axis=0))
                pos_sb = act.tile([P, D], f32, tag="emb", bufs=2, name=f"pos{tc2}")
                nc.sync.dma_start(pos_sb[:], pos[tc2 * P:(tc2 + 1) * P, :])
                nc.vector.tensor_add(gat[:], gat[:], pos_sb[:])
                for dt in range(ND):
                    tp = ps.tile([P, P], f32, tag="att_s", bufs=4)
                    nc.tensor.transpose(tp[:], gat[:, dt * P:(dt + 1) * P], ident[:])
                    nc.vector.tensor_copy(x_sb[:, dt, tc2 * P:(tc2 + 1) * P], tp[:])

            # ---------------- layers ----------------
            for l in range(n_layers):
                parms_t = par.tile([P, NPAR], f32, tag="lnp")
                nc.sync.dma_start(parms_t[:], parms[l])

                h_sb = act.tile([P, ND, 256], bf16, tag="h", bufs=2)
                _ln(nc, ps, act, rows, cons, x_sb, h_sb)

                q_all = act.tile([P, 8, 256], bf16, tag="q")
                k_loc = act.tile([P, 8, 256], bf16, tag="kloc")
                # K first so the AllGather can start as early as possible
                for kf in range(8):
                    wb = wstr.tile([P, ND, P], bf16, tag="wb", bufs=6,
                                   name=f"wk{l}_{kf}")
                    nc.sync.dma_start(
                        wb[:].rearrange("p a b -> p (a b)"), qk_wb[l, kf])
                    acc = ps.tile([P, 256], f32, tag="acc")
                    for dt in range(ND):
                        nc.tensor.matmul(acc[:], lhsT=wb[:, dt, :],
                                         rhs=h_sb[:, dt, :],
                                         start=(dt == 0), stop=(dt == ND - 1))
                    nc.vector.tensor_scalar_add(k_loc[:, kf, :], acc[:],
                                                parms_t[:, 8 + kf:8 + kf + 1])

                # V (token-major, per head 65 cols = [v_h | 1])
                v_loc = act.tile([P, 2, 16 * 65], bf16, tag="vloc")
                v_loc_h = v_loc.rearrange("p c (h g) -> p c h g", h=16, g=65)
                for nb in range(2):
                    accs = [ps.tile([P, 512], f32, tag="av", bufs=2,
                                    name=f"vacc{l}_{nb}_{tc2}")
                            for tc2 in range(2)]
                    for dt in range(ND):
                        wv = wstr.tile([P, 512], bf16, tag="wv", bufs=4,
                                       name=f"wv{l}_{nb}_{dt}")
                        nc.sync.dma_start(wv[:], v_wb[l, nb, dt])
                        for tc2 in range(2):
                            nc.tensor.matmul(
                                accs[tc2][:],
                                lhsT=h_sb[:, dt, tc2 * P:(tc2 + 1) * P],
                                rhs=wv[:],
                                start=(dt == 0), stop=(dt == ND - 1))
                    for tc2 in range(2):
                        nc.vector.tensor_copy(
                            v_loc_h[:, tc2, nb * 8:(nb + 1) * 8, 0:64],
                            accs[tc2][:].rearrange("p (h g) -> p h g", h=8))
                for tc2 in range(2):
                    nc.vector.memset(v_loc_h[:, tc2, :, 64:65], 1.0)

                # K+V bounce split into two half AllGathers: half h carries
                # K j-tiles 4h:4h+4 and V heads 8h:8h+8, so pass-2 j=0..3 can
                # start after AG0 while AG1 is still in flight.
                kv_ins, kv_outs = [], []
                for hf in range(2):
                    kv_in = dram.tile([KVH], bf16, tag="kvin", name=f"kvin{l}_{hf}")
                    kv_out = dram.tile([4, KVH], bf16, tag="kvout",
                                       name=f"kvout{l}_{hf}")
                    nc.sync.dma_start(
                        kv_in[0:KH].rearrange("(f p t) -> p f t", f=4, p=P),
                        k_loc[:, 4 * hf:4 * hf + 4, :])
                    for tc2 in range(2):
                        nc.sync.dma_start(
                            kv_in[KH + tc2 * P * 520:KH + (tc2 + 1) * P * 520]
                            .rearrange("(p f) -> p f", p=P),
                            v_loc[:, tc2, 520 * hf:520 * (hf + 1)])
                    if single:
                        nc.sync.dma_start(kv_out[0], kv_in[:])
                    else:
                        nc.gpsimd.collective_compute(
                            "AllGather", OP.bypass, replica_groups=kv_groups,
                            ins=[kv_in.opt()], outs=[kv_out.opt()])
                    kv_ins.append(kv_in)
                    kv_outs.append(kv_out)

                # Q projections (overlap the AllGather)
                for qf in range(8):
                    wb = wstr.tile([P, ND, P], bf16, tag="wb", bufs=6,
                                   name=f"wq{l}_{qf}")
                    nc.sync.dma_start(
                        wb[:].rearrange("p a b -> p (a b)"), qk_wb[l, 8 + qf])
                    acc = ps.tile([P, 256], f32, tag="acc")
                    for dt in range(ND):
                        nc.tensor.matmul(acc[:], lhsT=wb[:, dt, :],
                                         rhs=h_sb[:, dt, :],
                                         start=(dt == 0), stop=(dt == ND - 1))
                    nc.vector.tensor_scalar_add(q_all[:, qf, :], acc[:],
                                                parms_t[:, qf:qf + 1])

                o_sb = act.tile([P, ND, 256], bf16, tag="o")
                scale = 1.0 / np.sqrt(DH)

                # Pass 1 (pre-AllGather): attention over this core's OWN 256
                # k-tokens; both k-chunks share one PSUM bank -> one exp each.
                o_own = {}
                for j in range(8):
                    for hh in range(2):
                        h_idx = 2 * j + hh
                        base = hh * 64
                        sps = ps.tile([P, 512], f32, tag="att_s", bufs=4,
                                      name=f"spp{l}_{j}_{hh}")
                        for c in range(2):
                            nc.tensor.matmul(
                                sps[:, c * 256:(c + 1) * 256],
                                lhsT=k_loc[base:base + 64, j, c * P:(c + 1) * P],
                                rhs=q_all[base:base + 64, j, :],
                                start=(c == 0), stop=True)
                        e = act.tile([P, 512], bf16, tag="e", bufs=4,
                                     name=f"ep{l}_{j}_{hh}")
                        nc.scalar.activation(e[:], sps[:], AF.Exp, scale=scale)
                        avp = ps.tile([P, 512], f32, tag="av", bufs=2,
                                      name=f"avp{l}_{j}_{hh}")
                        for c in range(2):
                            nc.tensor.matmul(
                                avp[0:65, 0:256],
                                lhsT=v_loc_h[:, c, h_idx, :],
                                rhs=e[:, c * 256:(c + 1) * 256],
                                start=(c == 0), stop=(c == 1))
                        snap = act.tile([65, 256], bf16, tag="avown", bufs=16,
                                        name=f"oo{l}_{j}_{hh}")
                        nc.vector.tensor_copy(snap[:], avp[0:65, 0:256])
                        o_own[(j, hh)] = snap

                # Pass 2: stream the three OTHER ranks' K/V per j from the
                # gathered dram buffer; re-inject partial [o|Z]; head pairs
                # share PSUM banks -> one exp per chunk.
                k_halves, v_halves = [], []
                for hf in range(2):
                    k_h = act.tile([P, 4, 768], bf16, tag=f"ksba{hf}",
                                   name=f"kall{l}_{hf}")
                    for i in range(3):
                        nc.sync.dma_start(
                            k_h[:, :, i * 256:(i + 1) * 256],
                            kv_outs[hf][bass.ds(oth_ranks[i], 1), 0:KH]
                            .rearrange("o (f p t) -> p f (o t)", f=4, p=P))
                    v_h = act.tile([P, 6, 8 * 65], bf16, tag=f"vsba{hf}",
                                   name=f"vall{l}_{hf}")
                    for i in range(3):
                        nc.sync.dma_start(
                            v_h[:, 2 * i:2 * i + 2, :],
                            kv_outs[hf][bass.ds(oth_ranks[i], 1), KH:KVH]
                            .rearrange("o (th p f) -> p (o th) f", p=P,
                                       f=8 * 65))
                    k_halves.append(k_h)
                    v_halves.append(v_h.rearrange("p c (h g) -> p c h g",
                                                  h=8, g=65))
                for j in range(8):
                    k_j = k_halves[j // 4][:, j % 4, :]
                    v_jh = v_halves[j // 4]

                    av = ps.tile([P, 512], f32, tag="av", bufs=2,
                                 name=f"av{l}_{j}")
                    for hh in range(2):
                        # one start=True per bank tenancy: the 2nd region's
                        # has_written bits are already cleared by the 1st
                        nc.tensor.matmul(av[0:65, hh * 256:(hh + 1) * 256],
                                         lhsT=ident_bf[0:65, 0:65],
                                         rhs=o_own[(j, hh)][:],
                                         start=(hh == 0), stop=False)
                    # chunk-PAIRS share a PSUM bank (same lhsT row group ->
                    # serialized PE drains; packing the two head-halves
                    # instead would put different row groups on one bank =
                    # concurrent drains = PSUM collision)
                    for cp in range(3):
                        for hh in range(2):
                            base = hh * 64
                            sps = ps.tile([P, 512], f32, tag="att_s", bufs=4,
                                          name=f"sps{l}_{j}_{cp}_{hh}")
                            for ci in range(2):
                                c = 2 * cp + ci
                                nc.tensor.matmul(
                                    sps[:, ci * 256:(ci + 1) * 256],
                                    lhsT=k_j[base:base + 64, c * 128:(c + 1) * 128],
                                    rhs=q_all[base:base + 64, j, :],
                                    start=(ci == 0), stop=True)
                            e = act.tile([P, 512], bf16, tag="e", bufs=4,
                                         name=f"e{l}_{j}_{cp}_{hh}")
                            nc.scalar.activation(e[:], sps[:], AF.Exp, scale=scale)
                            for ci in range(2):
                                c = 2 * cp + ci
                                nc.tensor.matmul(
                                    av[0:65, hh * 256:(hh + 1) * 256],
                                    lhsT=v_jh[:, c, 2 * (j % 4) + hh, :],
                                    rhs=e[:, ci * 256:(ci + 1) * 256],
                                    start=False, stop=(c == 5))
                    recip = rows.tile([1, 512], f32, tag="row5", bufs=3)
                    nc.vector.reciprocal(recip[:], av[64:65, :])
                    bc_sb = act.tile([P, 512], f32, tag="bcsb", bufs=2)
                    nc.gpsimd.partition_broadcast(bc_sb[0:64, :], recip[:],
                                                  channels=64)
                    nc.vector.tensor_mul(o_sb[0:64, j, :], av[0:64, 0:256],
                                         bc_sb[0:64, 0:256])
                    o_st = act.tile([64, 256], bf16, tag="ost", bufs=2)
                    nc.vector.tensor_mul(o_st[:], av[0:64, 256:512],
                                         bc_sb[0:64, 256:512])
                    nc.sync.dma_start(o_sb[64:128, j, :], o_st[:])
                    # + v bias (sum of softmax weights == 1)
                    nc.vector.tensor_scalar_add(o_sb[:, j, :], o_sb[:, j, :],
                                                parms_t[:, 16 + j:16 + j + 1])

                # attention out-proj + residual
                for do in range(ND):
                    wb = wstr.tile([P, ND, P], bf16, tag="wb", bufs=6,
                                   name=f"wpj{l}_{do}")
                    nc.sync.dma_start(
                        wb[:].rearrange("p a b -> p (a b)"), pj_wb[l, do])
                    acc = ps.tile([P, 256], f32, tag="acc")
                    for dt in range(ND):
                        nc.tensor.matmul(acc[:], lhsT=wb[:, dt, :],
                                         rhs=o_sb[:, dt, :],
                                         start=(dt == 0), stop=(dt == ND - 1))
                    nc.vector.scalar_tensor_tensor(
                        out=x_sb[:, do, :], in0=acc[:],
                        scalar=parms_t[:, 24 + do:24 + do + 1],
                        in1=x_sb[:, do, :], op0=OP.add, op1=OP.add)

                # LN2 + MLP (fc1 -> gelu -> fc2 interleaved ft-outer;
                # fc2 accumulates into 4 [P,512] banks = 8 d-outs)
                h2_sb = act.tile([P, ND, 256], bf16, tag="h", bufs=2)
                _ln(nc, ps, act, rows, cons, x_sb, h2_sb)

                f2accs = [ps.tile([P, 512], f32, tag="att_s", bufs=4,
                                  name=f"f2acc{l}_{b}") for b in range(4)]
                for ft in range(NFT):
                    wb = wstr.tile([P, ND, P], bf16, tag="wb", bufs=6,
                                   name=f"wf1{l}_{ft}")
                    nc.sync.dma_start(
                        wb[:].rearrange("p a b -> p (a b)"), f1_wb[l, ft])
                    acc = ps.tile([P, 256], f32, tag="acc")
                    for dt in range(ND):
                        nc.tensor.matmul(acc[:], lhsT=wb[:, dt, :],
                                         rhs=h2_sb[:, dt, :],
                                         start=(dt == 0), stop=(dt == ND - 1))
                    h1 = act.tile([P, 256], bf16, tag="h1", bufs=6,
                                  name=f"h1{l}_{ft}")
                    nc.scalar.activation(h1[:], acc[:], AF.Gelu,
                                         bias=parms_t[:, 32 + ft:32 + ft + 1])
                    w2 = wstr.tile([P, D], bf16, tag="wf2", bufs=4,
                                   name=f"wf2{l}_{ft}")
                    nc.sync.dma_start(w2[:], f2_wb[l, ft])
                    for do in range(ND):
                        # start only on each bank's FIRST region (start=True
                        # clears has_written for the whole bank)
                        nc.tensor.matmul(
                            f2accs[do // 2][:, (do % 2) * 256:(do % 2 + 1) * 256],
                            lhsT=w2[:, do * P:(do + 1) * P], rhs=h1[:],
                            start=(ft == 0 and do % 2 == 0), stop=(ft == NFT - 1))
                for do in range(ND):
                    nc.vector.scalar_tensor_tensor(
                        out=x_sb[:, do, :],
                        in0=f2accs[do // 2][:, (do % 2) * 256:(do % 2 + 1) * 256],
                        scalar=parms_t[:, 64 + do:64 + do + 1],
                        in1=x_sb[:, do, :], op0=OP.add, op1=OP.add)

            # ---------------- final LN + AllGather + LM head ----------------
            xf_sb = act.tile([P, ND, 256], bf16, tag="h", bufs=2)
            _ln(nc, ps, act, rows, cons, x_sb, xf_sb)

            xf_in = dram.tile([ND, P, 256], bf16)
            xf_out = dram.tile([NCORES, ND, P, 256], bf16, addr_space="Shared")
            nc.sync.dma_start(xf_in.rearrange("d p t -> p d t"), xf_sb[:])
            if single:
                nc.sync.dma_start(xf_out[0], xf_in[:])
            else:
                nc.gpsimd.collective_compute(
                    "AllGather", OP.bypass, replica_groups=all_group,
                    ins=[xf_in.opt()], outs=[xf_out.opt()])

            # release layer-phase pools; LM phase gets all 8 PSUM banks
            act.release()
            ps.release()
            lmact = tc.alloc_tile_pool(name="lmact", bufs=1)
            psB = tc.alloc_tile_pool(name="psB", bufs=8, space="PSUM")

            xall = []
            for g in range(4):
                xt = lmact.tile([P, 2, NT], bf16, tag="xall", bufs=4,
                                name=f"xall{g}")
                for i in range(2):
                    dt = 2 * g + i
                    nc.sync.dma_start(
                        xt[:, i, :].rearrange("p (r t) -> p r t", r=NCORES),
                        xf_out[:, dt, :, :].rearrange("r p t -> p r t"))
                xall.append(xt)
            lmw = []
            for dt in range(ND):
                wt = lmact.tile([P, VSP], bf16, tag="lmw", bufs=8,
                                name=f"lmw{dt}")
                nc.sync.dma_start(wt[:], lm_wT[dt * P:(dt + 1) * P, :])
                lmw.append(wt)
            lmb_row = rows.tile([1, VSP], f32, tag="lmbrow", bufs=1)
            nc.sync.dma_start(lmb_row[:], lm_b[None, :])
            lmb_bc = lmact.tile([P, VSP], f32, tag="lmbbc")
            nc.gpsimd.partition_broadcast(lmb_bc[:], lmb_row[:], channels=P)

            for tk in range(NT // P):
                accs = [psB.tile([P, 512], f32, tag="lmacc", name=f"lmacc{tk}_{v}")
                        for v in range(8)]
                for dt in range(ND):
                    lhs = xall[dt // 2][:, dt % 2, tk * P:(tk + 1) * P]
                    for vc in range(8):
                        nc.tensor.matmul(
                            accs[vc][:], lhsT=lhs,
                            rhs=lmw[dt][:, vc * 512:(vc + 1) * 512],
                            start=(dt == 0), stop=(dt == ND - 1))
                for vc in range(8):
                    osb = lmact.tile([P, 512], f32, tag="osb", bufs=4)
                    nc.vector.tensor_add(osb[:], accs[vc][:],
                                         lmb_bc[:, vc * 512:(vc + 1) * 512])
                    nc.sync.dma_start(
                        out_tok[tk * P:(tk + 1) * P, vc * 512:(vc + 1) * 512],
                        osb[:])
            lmact.release()
            psB.release()

    nc.compile()
    return nc


def _prep_in_maps(inputs, n_layers=L):
    input_ids = np.asarray(inputs["input_ids"]).reshape(NT).astype(np.int32)
    pos_w = np.asarray(inputs["pos_w"], dtype=np.float32)
    embed_w = np.ascontiguousarray(np.asarray(inputs["embed_w"], dtype=np.float32))

    f = np.float32
    attn_in_w = np.asarray(inputs["attn_in_w"], f)    # [L, 3D, D]
    attn_in_b = np.asarray(inputs["attn_in_b"], f)    # [L, 3D]
    ln1_s = np.asarray(inputs["ln1_s"], f)
    ln1_b = np.asarray(inputs["ln1_b"], f)
    fc1_w = np.asarray(inputs["fc1_w"], f)            # [L, F, D]
    fc1_b = np.asarray(inputs["fc1_b"], f)
    ln2_s = np.asarray(inputs["ln2_s"], f)
    ln2_b = np.asarray(inputs["ln2_b"], f)
    fc2_w = np.asarray(inputs["fc2_w"], f)            # [L, D, F]
    fc2_b = np.asarray(inputs["fc2_b"], f)
    proj_w = np.asarray(inputs["attn_out_w"], f)      # [L, D, D]
    proj_b = np.asarray(inputs["attn_out_b"], f)
    lm_w = np.asarray(inputs["lm_w"], f)
    lm_b_full = np.asarray(inputs["lm_b"], f)
    lnf_s = np.asarray(inputs["lnf_s"], f)
    lnf_b = np.asarray(inputs["lnf_b"], f)

    # fold LN scale into the next matmul's weights, LN bias into its bias
    attn_w_f = attn_in_w * ln1_s[:, None, :]          # [L, 3D, D]
    attn_b_f = attn_in_b + np.einsum("led,ld->le", attn_in_w, ln1_b)
    fc1_w_f = fc1_w * ln2_s[:, None, :]
    fc1_b_f = fc1_b + np.einsum("lfd,ld->lf", fc1_w, ln2_b)
    lm_w_f = lm_w * lnf_s[None, :]
    lm_b_f = lm_b_full + lm_w @ lnf_b

    bf = ml_dtypes.bfloat16
    # All lhsT bundles carry the CONTRACTION dim on partitions:
    # bundle[l, ftile, p, dt*128 + c] = W^T[dt*128 + p, ftile*128 + c].
    awT = np.transpose(attn_w_f, (0, 2, 1))           # [L, D(in), 3D(out)]
    qk_full = awT.reshape(L, ND, P, 24, P).transpose(0, 3, 2, 1, 4).reshape(
        L, 24, P, D)                                  # [L, ftile, p_in, dt*c]
    qk_order = np.concatenate([np.arange(8, 16), np.arange(0, 8)])  # k then q
    qk_wb = np.ascontiguousarray(qk_full[:, qk_order]).astype(bf)
    # v weights (moving operand): [L, 2nb, 8dt, 128 d_in_row, 512 vfeat]
    vwT = awT[:, :, 2 * D:3 * D]                      # [L, D(in), 1024 vf]
    v_wb = np.ascontiguousarray(
        vwT.reshape(L, ND, P, 2, 512).transpose(0, 3, 1, 2, 4)).astype(bf)
    # proj bundles: [L, 8do, 128 p_in, 8dt*128 out]
    pjT = np.transpose(proj_w, (0, 2, 1))             # [L, D(in), D(out)]
    pj_wb = np.ascontiguousarray(
        pjT.reshape(L, ND, P, ND, P).transpose(0, 3, 2, 1, 4).reshape(
            L, ND, P, D)).astype(bf)
    # fc1 bundles: [L, 32ft, 128 p_in, 8dt*128]
    f1T = np.transpose(fc1_w_f, (0, 2, 1))            # [L, D(in), F(out)]
    f1_wb = np.ascontiguousarray(
        f1T.reshape(L, ND, P, NFT, P).transpose(0, 3, 2, 1, 4).reshape(
            L, NFT, P, D)).astype(bf)
    # fc2 per-ft: [L, 32ft(in rows), 128 f_in_row, 1024 d_out]
    f2_wb = np.ascontiguousarray(
        np.transpose(fc2_w, (0, 2, 1)).reshape(L, NFT, P, D)).astype(bf)

    parms = np.zeros((L, P, NPAR), f)
    parms[:, :, 0:24] = attn_b_f.reshape(L, 24, P).transpose(0, 2, 1)
    parms[:, :, 24:32] = proj_b.reshape(L, 8, P).transpose(0, 2, 1)
    parms[:, :, 32:64] = fc1_b_f.reshape(L, 32, P).transpose(0, 2, 1)
    parms[:, :, 64:72] = fc2_b.reshape(L, 8, P).transpose(0, 2, 1)

    common = {
        "embed_w": embed_w,
        "qk_wb": qk_wb,
        "v_wb": v_wb,
        "pj_wb": pj_wb,
        "f1_wb": f1_wb,
        "f2_wb": f2_wb,
        "parms": parms,
    }

    in_maps = []
    for c in range(NCORES):
        s0 = (c % 4) * T
        lm_shard = np.zeros((VSP, D), f)
        lm_shard[:VS] = lm_w_f[c * VS:(c + 1) * VS]
        lmb_shard = np.zeros(VSP, f)
        lmb_shard[:VS] = lm_b_f[c * VS:(c + 1) * VS]
        m = dict(common)
        m["ids"] = input_ids[c * T:(c + 1) * T]
        m["pos"] = np.ascontiguousarray(pos_w[s0:s0 + T])
        m["lm_wT"] = np.ascontiguousarray(lm_shard.T).astype(bf)
        m["lm_b"] = lmb_shard
        in_maps.append(m)
    return in_maps


def _assemble(results):
    parts = [results[c]["out_tok"][:, :VS] for c in range(NCORES)]
    logits = np.concatenate(parts, axis=1)     # [2048, 32000]
    return np.ascontiguousarray(logits.reshape(B, S, V).astype(np.float32))


_NC_CACHE = {}


def _get_nc(n_layers=L):
    if n_layers not in _NC_CACHE:
        _NC_CACHE[n_layers] = build(n_layers)
    return _NC_CACHE[n_layers]


def run(inputs, n_layers=L, trace=False, trace_cores=None):
    nc = _get_nc(n_layers)
    in_maps = _prep_in_maps(inputs, n_layers)
    if trace:
        try:
            import axon_ntff_shim
            axon_ntff_shim.install()
        except Exception:
            pass
    res = bass_utils.run_bass_kernel_spmd(
        nc, in_maps, core_ids=list(range(NCORES)), trace=trace,
        trace_cores=(trace_cores or [0]) if trace else None)
    return _assemble(res.results), res


def kernel(**inputs) -> np.ndarray:
    out, _ = run(inputs)
    return out


# revision 5
# speedup vs baseline: 1.1204x; 1.0104x over previous
"""Trainium2 Bass kernel for a 4-layer transformer (B=2,S=1024,D=1024,H=16,F=4096,V=32000).

Strategy (8 NeuronCores): sequence-parallel layers (256 tokens/core, weights
replicated, streamed bf16), feature-major activations, per-layer merged K+V
AllGather within each batch's 4-core group, vocab-sharded LM head.

v2 changes vs baseline:
 - Weights streamed as per-output-tile bundles ([128, 8dt, 128] host-packed)
   through small rotating SBUF slots instead of 8 resident 1MB tiles; fc2 is
   consumed ft-outer right behind fc1+gelu, accumulating into 8 PSUM banks.
 - LN scale/bias folded into the following weights on host; device LN is pure
   standardize with DVE Newton-rsqrt (ACT only runs Exp + Gelu).
 - All small per-layer params packed into one [128, 72] tensor (one DMA).
 - K and V gathered in ONE AllGather per layer; gathered K/V streamed per-j.
 - Attention scores packed 2-per-PSUM-bank -> exp on [128,512] tiles.
 - PSUM evacuations/bias-adds moved from ACT to DVE.
"""
import numpy as np
import ml_dtypes

import concourse.bass as bass
import concourse.bacc as bacc
import concourse.mybir as mybir
import concourse.tile as tile
from concourse import bass_utils
from concourse.masks import make_identity

B, S, D, H, L, F, V = 2, 1024, 1024, 16, 4, 4096, 32000
DH = D // H          # 64
NCORES = 8
T = (B * S) // NCORES  # 256 tokens per core
NT = B * S             # 2048
VS = V // NCORES       # 4000
VSP = 4096             # padded vocab shard
P = 128
ND = D // P            # 8 d-tiles
NFT = F // P           # 32 fc1 f-tiles
NPAR = 72              # packed params: qkvb[0:24] projb[24:32] fc1b[32:64] fc2b[64:72]
KSZ = 8 * P * 256          # k elems in merged AG buffer
VSZ = 256 * 16 * 65        # v elems ([2 th, 128 p, 16h*65])
KVTOT = KSZ + VSZ
KH = 4 * P * 256           # k elems per half (4 j-tiles)
VH = 256 * 8 * 65          # v elems per half (8 heads)
KVH = KH + VH              # one half-AG payload

f32 = mybir.dt.float32
bf16 = mybir.dt.bfloat16
f8 = mybir.dt.float8e4
i32 = mybir.dt.int32
u32 = mybir.dt.uint32
AF = mybir.ActivationFunctionType
OP = mybir.AluOpType

RSQRT_MAGIC_P1 = 0x5F3759E0  # 0x5f3759df + 1 (two's-complement sub via xor+add)


def _ln(nc, ps, act, rows, cons, x_sb, out_h):
    """Standardize: x_sb [128, 8, 256] f32 -> out_h [128, 8, 256] bf16.

    (x - mu) * rsqrt(var + eps); LN scale/bias are folded into the next
    weight matrix on the host. Stats via paired N=512 ones-matmuls; rsqrt
    via DVE bit-hack seed + 2 Newton steps (ACT tables stay on exp/gelu)."""
    stat_x = ps.tile([1, 512], f32, tag="att_s", bufs=4)
    stat_q = ps.tile([1, 512], f32, tag="att_s", bufs=4)
    for dp in range(4):
        xp = x_sb[:, 2 * dp:2 * dp + 2, :].rearrange("p a b -> p (a b)")
        sq = act.tile([P, 512], f32, tag="sq", bufs=2)
        nc.vector.tensor_mul(sq[:], xp, xp)
        nc.tensor.matmul(stat_x[:], lhsT=cons.ones_col_f32[:], rhs=xp,
                         start=(dp == 0), stop=(dp == 3))
        nc.tensor.matmul(stat_q[:], lhsT=cons.ones_col_f32[:], rhs=sq[:],
                         start=(dp == 0), stop=(dp == 3))
    murow = rows.tile([1, 512], f32, tag="row")  # [mu | invstd]
    sxrow = rows.tile([1, 512], f32, tag="row")
    nc.vector.tensor_copy(sxrow[:], stat_x[:])
    sqrow = rows.tile([1, 512], f32, tag="row")
    nc.vector.tensor_copy(sqrow[:], stat_q[:])
    sumx = rows.tile([1, 256], f32, tag="row")
    nc.vector.tensor_add(sumx[:], sxrow[:, 0:256], sxrow[:, 256:512])
    nc.vector.tensor_scalar(murow[:, 0:256], sumx[:], 1.0 / D, None, OP.mult)
    msq = rows.tile([1, 256], f32, tag="row")
    nc.vector.tensor_add(msq[:], sqrow[:, 0:256], sqrow[:, 256:512])
    nc.vector.tensor_scalar(msq[:], msq[:], 1.0 / D, 1e-5, OP.mult, OP.add)
    mu2 = rows.tile([1, 256], f32, tag="row")
    nc.vector.tensor_mul(mu2[:], murow[:, 0:256], murow[:, 0:256])
    vare = rows.tile([1, 256], f32, tag="row")
    nc.vector.tensor_sub(vare[:], msq[:], mu2[:])
    std = rows.tile([1, 256], f32, tag="row")
    nc.scalar.activation(std[:], vare[:], AF.Sqrt)
    nc.vector.reciprocal(murow[:, 256:512], std[:])
    bc = act.tile([P, 512], f32, tag="lnbc", bufs=2)
    nc.gpsimd.partition_broadcast(bc[:], murow[:], channels=P)
    for dt in range(ND):
        t = act.tile([P, 256], f32, tag="lnt", bufs=3)
        nc.vector.tensor_sub(t[:], x_sb[:, dt, :], bc[:, 0:256])
        nc.vector.tensor_mul(out_h[:, dt, :], t[:], bc[:, 256:512])


class _Cons:
    pass


def build(n_layers=L, single=False):
    """single=True: 1-core variant with collectives replaced by local DMA
    copies (for TimelineSim cost-model analysis only — wrong numerics)."""
    nc = bacc.Bacc("TRN2", target_bir_lowering=False, debug=False,
                   num_devices=1 if single else NCORES)

    ids = nc.dram_tensor("ids", [T], i32, kind="ExternalInput").ap()
    pos = nc.dram_tensor("pos", [T, D], f32, kind="ExternalInput").ap()
    embed_w = nc.dram_tensor("embed_w", [V, D], f32, kind="ExternalInput").ap()
    # qk bundles: [L, 16, 128, 8dt*128] — k fts first (8), then q fts (8)
    qk_wb = nc.dram_tensor("qk_wb", [L, 16, P, ND * P], bf16, kind="ExternalInput").ap()
    # v weight stream: [L, 2nb, 8dt, 128, 512]
    v_wb = nc.dram_tensor("v_wb", [L, 2, ND, P, 512], bf16, kind="ExternalInput").ap()
    # proj bundles: [L, 8do, 128, 8dt*128]
    pj_wb = nc.dram_tensor("pj_wb", [L, ND, P, ND * P], bf16, kind="ExternalInput").ap()
    # fc1 bundles: [L, 32ft, 128, 8dt*128]
    f1_wb = nc.dram_tensor("f1_wb", [L, NFT, P, ND * P], bf16, kind="ExternalInput").ap()
    # fc2 per-ft rows: [L, 32ft, 128, 1024]
    f2_wb = nc.dram_tensor("f2_wb", [L, NFT, P, D], bf16, kind="ExternalInput").ap()
    parms = nc.dram_tensor("parms", [L, P, NPAR], f32, kind="ExternalInput").ap()
    lm_wT = nc.dram_tensor("lm_wT", [D, VSP], bf16, kind="ExternalInput").ap()
    lm_b = nc.dram_tensor("lm_b", [VSP], f32, kind="ExternalInput").ap()
    out_tok = nc.dram_tensor("out_tok", [NT, VSP], f32, kind="ExternalOutput").ap()

    kv_groups = [[0, 1, 2, 3], [4, 5, 6, 7]]
    all_group = [list(range(NCORES))]

    with tile.TileContext(nc) as tc:
        with (
            tc.tile_pool(name="consp", bufs=1) as consp,
            tc.tile_pool(name="wstr", bufs=1) as wstr,
            tc.tile_pool(name="rows", bufs=6) as rows,
            tc.tile_pool(name="par", bufs=2) as par,
            tc.tile_pool(name="dram", bufs=1, space="DRAM") as dram,
        ):
            act = tc.alloc_tile_pool(name="act", bufs=1)
            ps = tc.alloc_tile_pool(name="ps", bufs=2, space="PSUM")
            cons = _Cons()
            ident = consp.tile([P, P], f32)
            make_identity(nc, ident)
            ident_bf = consp.tile([P, P], bf16)
            nc.vector.tensor_copy(ident_bf[:], ident[:])
            ones_col_f32 = consp.tile([P, 1], f32)
            nc.vector.memset(ones_col_f32[:], 1.0)
            cons.ones_col_f32 = ones_col_f32

            x_sb = consp.tile([P, ND, 256], f32)  # residual, feature-major

            # per-core group-rank registers for own-block-skipping dynamic DMAs
            seng = nc.sync
            pid = seng.partition_id()
            rgrp = seng.alloc_register("grp_rank")
            seng.reg_alu(rgrp, pid, 3, OP.bitwise_and)
            grp_rank = seng.snap(rgrp, donate=True, min_val=0, max_val=3)
            oth_ranks = []
            for i in range(3):
                ra = seng.alloc_register(f"oth{i}a")
                seng.reg_alu(ra, grp_rank, i + 1, OP.add)
                rb = seng.alloc_register(f"oth{i}b")
                seng.reg_alu(rb, ra, 3, OP.bitwise_and)
                oth_ranks.append(seng.snap(rb, donate=True, min_val=0, max_val=3))

            # ---------------- embedding ----------------
            for tc2 in range(2):
                ids_sb = par.tile([P, 1], i32, tag="ids")
                nc.sync.dma_start(ids_sb[:], ids[tc2 * P:(tc2 + 1) * P, None])
                gat = act.tile([P, D], f32, tag="emb", bufs=2, name=f"gat{tc2}")
                nc.gpsimd.indirect_dma_start(
                    out=gat[:], out_offset=None, in_=embed_w[:],
                    in_offset=bass.IndirectOffsetOnAxis(ap=ids_sb[:, :1], axis=0))
                pos_sb = act.tile([P, D], f32, tag="emb", bufs=2, name=f"pos{tc2}")
                nc.sync.dma_start(pos_sb[:], pos[tc2 * P:(tc2 + 1) * P, :])
                nc.vector.tensor_add(gat[:], gat[:], pos_sb[:])
                for dt in range(ND):
                    tp = ps.tile([P, P], f32, tag="att_s", bufs=4)
                    nc.tensor.transpose(tp[:], gat[:, dt * P:(dt + 1) * P], ident[:])
                    nc.vector.tensor_copy(x_sb[:, dt, tc2 * P:(tc2 + 1) * P], tp[:])

            # ---------------- layers ----------------
            for l in range(n_layers):
                parms_t = par.tile([P, NPAR], f32, tag="lnp")
                nc.sync.dma_start(parms_t[:], parms[l])

                h_sb = act.tile([P, ND, 256], bf16, tag="h", bufs=2)
                _ln(nc, ps, act, rows, cons, x_sb, h_sb)

                q_all = act.tile([P, 8, 256], bf16, tag="q")
                k_loc = act.tile([P, 8, 256], bf16, tag="kloc")
                # K first so the AllGather can start as early as possible
                for kf in range(8):
                    wb = wstr.tile([P, ND, P], bf16, tag="wb", bufs=6,
                                   name=f"wk{l}_{kf}")
                    nc.sync.dma_start(
                        wb[:].rearrange("p a b -> p (a b)"), qk_wb[l, kf])
                    acc = ps.tile([P, 256], f32, tag="acc")
                    for dt in range(ND):
                        nc.tensor.matmul(acc[:], lhsT=wb[:, dt, :],
                                         rhs=h_sb[:, dt, :],
                                         start=(dt == 0), stop=(dt == ND - 1))
                    nc.vector.tensor_scalar_add(k_loc[:, kf, :], acc[:],
                                                parms_t[:, 8 + kf:8 + kf + 1])

                # V (token-major, per head 65 cols = [v_h | 1])
                v_loc = act.tile([P, 2, 16 * 65], bf16, tag="vloc")
                v_loc_h = v_loc.rearrange("p c (h g) -> p c h g", h=16, g=65)
                for nb in range(2):
                    accs = [ps.tile([P, 512], f32, tag="av", bufs=2,
                                    name=f"vacc{l}_{nb}_{tc2}")
                            for tc2 in range(2)]
                    for dt in range(ND):
                        wv = wstr.tile([P, 512], bf16, tag="wv", bufs=4,
                                       name=f"wv{l}_{nb}_{dt}")
                        nc.sync.dma_start(wv[:], v_wb[l, nb, dt])
                        for tc2 in range(2):
                            nc.tensor.matmul(
                                accs[tc2][:],
                                lhsT=h_sb[:, dt, tc2 * P:(tc2 + 1) * P],
                                rhs=wv[:],
                                start=(dt == 0), stop=(dt == ND - 1))
                    for tc2 in range(2):
                        nc.vector.tensor_copy(
                            v_loc_h[:, tc2, nb * 8:(nb + 1) * 8, 0:64],
                            accs[tc2][:].rearrange("p (h g) -> p h g", h=8))
                for tc2 in range(2):
                    nc.vector.memset(v_loc_h[:, tc2, :, 64:65], 1.0)

                # K+V bounce split into two half AllGathers: half h carries
                # K j-tiles 4h:4h+4 and V heads 8h:8h+8, so pass-2 j=0..3 can
                # start after AG0 while AG1 is still in flight.
                kv_ins, kv_outs = [], []
                for hf in range(2):
                    kv_in = dram.tile([KVH], bf16, tag="kvin", name=f"kvin{l}_{hf}")
                    kv_out = dram.tile([4, KVH], bf16, tag="kvout",
                                       name=f"kvout{l}_{hf}")
                    nc.sync.dma_start(
                        kv_in[0:KH].rearrange("(f p t) -> p f t", f=4, p=P),
                        k_loc[:, 4 * hf:4 * hf + 4, :])
                    for tc2 in range(2):
                        nc.sync.dma_start(
                            kv_in[KH + tc2 * P * 520:KH + (tc2 + 1) * P * 520]
                            .rearrange("(p f) -> p f", p=P),
                            v_loc[:, tc2, 520 * hf:520 * (hf + 1)])
                    if single:
                        nc.sync.dma_start(kv_out[0], kv_in[:])
                    else:
                        nc.gpsimd.collective_compute(
                            "AllGather", OP.bypass, replica_groups=kv_groups,
                            ins=[kv_in.opt()], outs=[kv_out.opt()])
                    kv_ins.append(kv_in)
                    kv_outs.append(kv_out)

                # Q projections (overlap the AllGather)
                for qf in range(8):
                    wb = wstr.tile([P, ND, P], bf16, tag="wb", bufs=6,
                                   name=f"wq{l}_{qf}")
                    nc.sync.dma_start(
                        wb[:].rearrange("p a b -> p (a b)"), qk_wb[l, 8 + qf])
                    acc = ps.tile([P, 256], f32, tag="acc")
                    for dt in range(ND):
                        nc.tensor.matmul(acc[:], lhsT=wb[:, dt, :],
                                         rhs=h_sb[:, dt, :],
                                         start=(dt == 0), stop=(dt == ND - 1))
                    nc.vector.tensor_scalar_add(q_all[:, qf, :], acc[:],
                                                parms_t[:, qf:qf + 1])

                o_sb = act.tile([P, ND, 256], bf16, tag="o")
                scale = 1.0 / np.sqrt(DH)

                # Pass 1 (pre-AllGather): attention over this core's OWN 256
                # k-tokens; both k-chunks share one PSUM bank -> one exp each.
                o_own = {}
                for j in range(8):
                    for hh in range(2):
                        h_idx = 2 * j + hh
                        base = hh * 64
                        sps = ps.tile([P, 512], f32, tag="att_s", bufs=4,
                                      name=f"spp{l}_{j}_{hh}")
                        for c in range(2):
                            nc.tensor.matmul(
                                sps[:, c * 256:(c + 1) * 256],
                                lhsT=k_loc[base:base + 64, j, c * P:(c + 1) * P],
                                rhs=q_all[base:base + 64, j, :],
                                start=(c == 0), stop=True)
                        e = act.tile([P, 512], bf16, tag="e", bufs=4,
                                     name=f"ep{l}_{j}_{hh}")
                        nc.scalar.activation(e[:], sps[:], AF.Exp, scale=scale)
                        avp = ps.tile([P, 512], f32, tag="av", bufs=2,
                                      name=f"avp{l}_{j}_{hh}")
                        for c in range(2):
                            nc.tensor.matmul(
                                avp[0:65, 0:256],
                                lhsT=v_loc_h[:, c, h_idx, :],
                                rhs=e[:, c * 256:(c + 1) * 256],
                                start=(c == 0), stop=(c == 1))
                        snap = act.tile([65, 256], bf16, tag="avown", bufs=16,
                                        name=f"oo{l}_{j}_{hh}")
                        nc.vector.tensor_copy(snap[:], avp[0:65, 0:256])
                        o_own[(j, hh)] = snap

                # Pass 2: stream the three OTHER ranks' K/V per j from the
                # gathered dram buffer; re-inject partial [o|Z]; head pairs
                # share PSUM banks -> one exp per chunk.
                k_halves, v_halves = [], []
                for hf in range(2):
                    k_h = act.tile([P, 4, 768], bf16, tag=f"ksba{hf}",
                                   name=f"kall{l}_{hf}")
                    for i in range(3):
                        nc.sync.dma_start(
                            k_h[:, :, i * 256:(i + 1) * 256],
                            kv_outs[hf][bass.ds(oth_ranks[i], 1), 0:KH]
                            .rearrange("o (f p t) -> p f (o t)", f=4, p=P))
                    v_h = act.tile([P, 6, 8 * 65], bf16, tag=f"vsba{hf}",
                                   name=f"vall{l}_{hf}")
                    for i in range(3):
                        nc.sync.dma_start(
                            v_h[:, 2 * i:2 * i + 2, :],
                            kv_outs[hf][bass.ds(oth_ranks[i], 1), KH:KVH]
                            .rearrange("o (th p f) -> p (o th) f", p=P,
                                       f=8 * 65))
                    k_halves.append(k_h)
                    v_halves.append(v_h.rearrange("p c (h g) -> p c h g",
                                                  h=8, g=65))
                for j in range(8):
                    k_j = k_halves[j // 4][:, j % 4, :]
                    v_jh = v_halves[j // 4]

                    av = ps.tile([P, 512], f32, tag="av", bufs=2,
                                 name=f"av{l}_{j}")
                    for hh in range(2):
                        # one start=True per bank tenancy: the 2nd region's
                        # has_written bits are already cleared by the 1st
                        nc.tensor.matmul(av[0:65, hh * 256:(hh + 1) * 256],
                                         lhsT=ident_bf[0:65, 0:65],
                                         rhs=o_own[(j, hh)][:],
                                         start=(hh == 0), stop=False)
                    # chunk-PAIRS share a PSUM bank (same lhsT row group ->
                    # serialized PE drains; packing the two head-halves
                    # instead would put different row groups on one bank =
                    # concurrent drains = PSUM collision)
                    for cp in range(3):
                        for hh in range(2):
                            base = hh * 64
                            sps = ps.tile([P, 512], f32, tag="att_s", bufs=4,
                                          name=f"sps{l}_{j}_{cp}_{hh}")
                            for ci in range(2):
                                c = 2 * cp + ci
                                nc.tensor.matmul(
                                    sps[:, ci * 256:(ci + 1) * 256],
                                    lhsT=k_j[base:base + 64, c * 128:(c + 1) * 128],
                                    rhs=q_all[base:base + 64, j, :],
                                    start=(ci == 0), stop=True)
                            e = act.tile([P, 512], bf16, tag="e", bufs=4,
                                         name=f"e{l}_{j}_{cp}_{hh}")
                            nc.scalar.activation(e[:], sps[:], AF.Exp, scale=scale)
                            for ci in range(2):
                                c = 2 * cp + ci
                                nc.tensor.matmul(
                                    av[0:65, hh * 256:(hh + 1) * 256],
                                    lhsT=v_jh[:, c, 2 * (j % 4) + hh, :],
                                    rhs=e[:, ci * 256:(ci + 1) * 256],
                                    start=False, stop=(c == 5))
                    recip = rows.tile([1, 512], f32, tag="row5", bufs=3)
                    nc.vector.reciprocal(recip[:], av[64:65, :])
                    bc_sb = act.tile([P, 512], f32, tag="bcsb", bufs=2)
                    nc.gpsimd.partition_broadcast(bc_sb[0:64, :], recip[:],
                                                  channels=64)
                    nc.vector.tensor_mul(o_sb[0:64, j, :], av[0:64, 0:256],
                                         bc_sb[0:64, 0:256])
                    o_st = act.tile([64, 256], bf16, tag="ost", bufs=2)
                    nc.vector.tensor_mul(o_st[:], av[0:64, 256:512],
                                         bc_sb[0:64, 256:512])
                    nc.sync.dma_start(o_sb[64:128, j, :], o_st[:])
                    # + v bias (sum of softmax weights == 1)
                    nc.vector.tensor_scalar_add(o_sb[:, j, :], o_sb[:, j, :],
                                                parms_t[:, 16 + j:16 + j + 1])

                # attention out-proj + residual
                for do in range(ND):
                    wb = wstr.tile([P, ND, P], bf16, tag="wb", bufs=6,
                                   name=f"wpj{l}_{do}")
                    nc.sync.dma_start(
                        wb[:].rearrange("p a b -> p (a b)"), pj_wb[l, do])
                    acc = ps.tile([P, 256], f32, tag="acc")
                    for dt in range(ND):
                        nc.tensor.matmul(acc[:], lhsT=wb[:, dt, :],
                                         rhs=o_sb[:, dt, :],
                                         start=(dt == 0), stop=(dt == ND - 1))
                    nc.vector.scalar_tensor_tensor(
                        out=x_sb[:, do, :], in0=acc[:],
                        scalar=parms_t[:, 24 + do:24 + do + 1],
                        in1=x_sb[:, do, :], op0=OP.add, op1=OP.add)

                # LN2 + MLP (fc1 -> gelu -> fc2 interleaved ft-outer;
                # fc2 accumulates into 4 [P,512] banks = 8 d-outs)
                h2_sb = act.tile([P, ND, 256], bf16, tag="h", bufs=2)
                _ln(nc, ps, act, rows, cons, x_sb, h2_sb)

                f2accs = [ps.tile([P, 512], f32, tag="att_s", bufs=4,
                                  name=f"f2acc{l}_{b}") for b in range(4)]
                for ft in range(NFT):
                    wb = wstr.tile([P, ND, P], bf16, tag="wb", bufs=6,
                                   name=f"wf1{l}_{ft}")
                    nc.sync.dma_start(
                        wb[:].rearrange("p a b -> p (a b)"), f1_wb[l, ft])
                    acc = ps.tile([P, 256], f32, tag="acc")
                    for dt in range(ND):
                        nc.tensor.matmul(acc[:], lhsT=wb[:, dt, :],
                                         rhs=h2_sb[:, dt, :],
                                         start=(dt == 0), stop=(dt == ND - 1))
                    h1 = act.tile([P, 256], bf16, tag="h1", bufs=6,
                                  name=f"h1{l}_{ft}")
                    nc.scalar.activation(h1[:], acc[:], AF.Gelu,
                                         bias=parms_t[:, 32 + ft:32 + ft + 1])
                    w2 = wstr.tile([P, D], bf16, tag="wf2", bufs=4,
                                   name=f"wf2{l}_{ft}")
                    nc.sync.dma_start(w2[:], f2_wb[l, ft])
                    for do in range(ND):
                        # start only on each bank's FIRST region (start=True
                        # clears has_written for the whole bank)
                        nc.tensor.matmul(
                            f2accs[do // 2][:, (do % 2) * 256:(do % 2 + 1) * 256],
                            lhsT=w2[:, do * P:(do + 1) * P], rhs=h1[:],
                            start=(ft == 0 and do % 2 == 0), stop=(ft == NFT - 1))
                for do in range(ND):
                    nc.vector.scalar_tensor_tensor(
                        out=x_sb[:, do, :],
                        in0=f2accs[do // 2][:, (do % 2) * 256:(do % 2 + 1) * 256],
                        scalar=parms_t[:, 64 + do:64 + do + 1],
                        in1=x_sb[:, do, :], op0=OP.add, op1=OP.add)

            # ---------------- final LN + AllGather + LM head ----------------
            xf_sb = act.tile([P, ND, 256], bf16, tag="h", bufs=2)
            _ln(nc, ps, act, rows, cons, x_sb, xf_sb)

            # final AG split by token halves: LM starts on half-0 tk tiles
            # while half-1 is still in flight
            xf_outs = []
            for hf in range(2):
                xf_in = dram.tile([ND, P, 128], bf16, tag="xfin",
                                  name=f"xfin{hf}")
                xf_out = dram.tile([NCORES, ND, P, 128], bf16,
                                   addr_space="Shared", tag="xfout",
                                   name=f"xfout{hf}")
                nc.sync.dma_start(xf_in.rearrange("d p t -> p d t"),
                                  xf_sb[:, :, 128 * hf:128 * (hf + 1)])
                if single:
                    nc.sync.dma_start(xf_out[0], xf_in[:])
                else:
                    nc.gpsimd.collective_compute(
                        "AllGather", OP.bypass, replica_groups=all_group,
                        ins=[xf_in.opt()], outs=[xf_out.opt()])
                xf_outs.append(xf_out)

            # release layer-phase pools; LM phase gets all 8 PSUM banks
            act.release()
            ps.release()
            lmact = tc.alloc_tile_pool(name="lmact", bufs=1)
            psB = tc.alloc_tile_pool(name="psB", bufs=8, space="PSUM")

            xall = [[], []]
            for hf in range(2):
                for g in range(4):
                    xt = lmact.tile([P, 2, NT // 2], bf16, tag=f"xall{hf}",
                                    bufs=4, name=f"xall{hf}_{g}")
                    for i in range(2):
                        dt = 2 * g + i
                        nc.sync.dma_start(
                            xt[:, i, :].rearrange("p (r t) -> p r t",
                                                  r=NCORES),
                            xf_outs[hf][:, dt, :, :].rearrange(
                                "r p t -> p r t"))
                    xall[hf].append(xt)
            lmw = []
            for dt in range(ND):
                wt = lmact.tile([P, VSP], bf16, tag="lmw", bufs=8,
                                name=f"lmw{dt}")
                nc.sync.dma_start(wt[:], lm_wT[dt * P:(dt + 1) * P, :])
                lmw.append(wt)
            lmb_row = rows.tile([1, VSP], f32, tag="lmbrow", bufs=1)
            nc.sync.dma_start(lmb_row[:], lm_b[None, :])
            lmb_bc = lmact.tile([P, VSP], f32, tag="lmbbc")
            nc.gpsimd.partition_broadcast(lmb_bc[:], lmb_row[:], channels=P)

            for tk in [t for hf in range(2)
                       for t in range(hf, NT // P, 2)]:
                hf, rk = tk % 2, tk // 2
                accs = [psB.tile([P, 512], f32, tag="lmacc", name=f"lmacc{tk}_{v}")
                        for v in range(8)]
                for dt in range(ND):
                    lhs = xall[hf][dt // 2][:, dt % 2, rk * P:(rk + 1) * P]
                    for vc in range(8):
                        nc.tensor.matmul(
                            accs[vc][:], lhsT=lhs,
                            rhs=lmw[dt][:, vc * 512:(vc + 1) * 512],
                            start=(dt == 0), stop=(dt == ND - 1))
                for vc in range(8):
                    osb = lmact.tile([P, 512], f32, tag="osb", bufs=4)
                    nc.vector.tensor_add(osb[:], accs[vc][:],
                                         lmb_bc[:, vc * 512:(vc + 1) * 512])
                    nc.sync.dma_start(
                        out_tok[tk * P:(tk + 1) * P, vc * 512:(vc + 1) * 512],
                        osb[:])
            lmact.release()
            psB.release()

    nc.compile()
    return nc


def _prep_in_maps(inputs, n_layers=L):
    input_ids = np.asarray(inputs["input_ids"]).reshape(NT).astype(np.int32)
    pos_w = np.asarray(inputs["pos_w"], dtype=np.float32)
    embed_w = np.ascontiguousarray(np.asarray(inputs["embed_w"], dtype=np.float32))

    f = np.float32
    attn_in_w = np.asarray(inputs["attn_in_w"], f)    # [L, 3D, D]
    attn_in_b = np.asarray(inputs["attn_in_b"], f)    # [L, 3D]
    ln1_s = np.asarray(inputs["ln1_s"], f)
    ln1_b = np.asarray(inputs["ln1_b"], f)
    fc1_w = np.asarray(inputs["fc1_w"], f)            # [L, F, D]
    fc1_b = np.asarray(inputs["fc1_b"], f)
    ln2_s = np.asarray(inputs["ln2_s"], f)
    ln2_b = np.asarray(inputs["ln2_b"], f)
    fc2_w = np.asarray(inputs["fc2_w"], f)            # [L, D, F]
    fc2_b = np.asarray(inputs["fc2_b"], f)
    proj_w = np.asarray(inputs["attn_out_w"], f)      # [L, D, D]
    proj_b = np.asarray(inputs["attn_out_b"], f)
    lm_w = np.asarray(inputs["lm_w"], f)
    lm_b_full = np.asarray(inputs["lm_b"], f)
    lnf_s = np.asarray(inputs["lnf_s"], f)
    lnf_b = np.asarray(inputs["lnf_b"], f)

    # fold LN scale into the next matmul's weights, LN bias into its bias
    attn_w_f = attn_in_w * ln1_s[:, None, :]          # [L, 3D, D]
    attn_b_f = attn_in_b + np.einsum("led,ld->le", attn_in_w, ln1_b)
    fc1_w_f = fc1_w * ln2_s[:, None, :]
    fc1_b_f = fc1_b + np.einsum("lfd,ld->lf", fc1_w, ln2_b)
    lm_w_f = lm_w * lnf_s[None, :]
    lm_b_f = lm_b_full + lm_w @ lnf_b

    bf = ml_dtypes.bfloat16
    # All lhsT bundles carry the CONTRACTION dim on partitions:
    # bundle[l, ftile, p, dt*128 + c] = W^T[dt*128 + p, ftile*128 + c].
    awT = np.transpose(attn_w_f, (0, 2, 1))           # [L, D(in), 3D(out)]
    qk_full = awT.reshape(L, ND, P, 24, P).transpose(0, 3, 2, 1, 4).reshape(
        L, 24, P, D)                                  # [L, ftile, p_in, dt*c]
    qk_order = np.concatenate([np.arange(8, 16), np.arange(0, 8)])  # k then q
    qk_wb = np.ascontiguousarray(qk_full[:, qk_order]).astype(bf)
    # v weights (moving operand): [L, 2nb, 8dt, 128 d_in_row, 512 vfeat]
    vwT = awT[:, :, 2 * D:3 * D]                      # [L, D(in), 1024 vf]
    v_wb = np.ascontiguousarray(
        vwT.reshape(L, ND, P, 2, 512).transpose(0, 3, 1, 2, 4)).astype(bf)
    # proj bundles: [L, 8do, 128 p_in, 8dt*128 out]
    pjT = np.transpose(proj_w, (0, 2, 1))             # [L, D(in), D(out)]
    pj_wb = np.ascontiguousarray(
        pjT.reshape(L, ND, P, ND, P).transpose(0, 3, 2, 1, 4).reshape(
            L, ND, P, D)).astype(bf)
    # fc1 bundles: [L, 32ft, 128 p_in, 8dt*128]
    f1T = np.transpose(fc1_w_f, (0, 2, 1))            # [L, D(in), F(out)]
    f1_wb = np.ascontiguousarray(
        f1T.reshape(L, ND, P, NFT, P).transpose(0, 3, 2, 1, 4).reshape(
            L, NFT, P, D)).astype(bf)
    # fc2 per-ft: [L, 32ft(in rows), 128 f_in_row, 1024 d_out]
    f2_wb = np.ascontiguousarray(
        np.transpose(fc2_w, (0, 2, 1)).reshape(L, NFT, P, D)).astype(bf)

    parms = np.zeros((L, P, NPAR), f)
    parms[:, :, 0:24] = attn_b_f.reshape(L, 24, P).transpose(0, 2, 1)
    parms[:, :, 24:32] = proj_b.reshape(L, 8, P).transpose(0, 2, 1)
    parms[:, :, 32:64] = fc1_b_f.reshape(L, 32, P).transpose(0, 2, 1)
    parms[:, :, 64:72] = fc2_b.reshape(L, 8, P).transpose(0, 2, 1)

    common = {
        "embed_w": embed_w,
        "qk_wb": qk_wb,
        "v_wb": v_wb,
        "pj_wb": pj_wb,
        "f1_wb": f1_wb,
        "f2_wb": f2_wb,
        "parms": parms,
    }

    in_maps = []
    for c in range(NCORES):
        s0 = (c % 4) * T
        lm_shard = np.zeros((VSP, D), f)
        lm_shard[:VS] = lm_w_f[c * VS:(c + 1) * VS]
        lmb_shard = np.zeros(VSP, f)
        lmb_shard[:VS] = lm_b_f[c * VS:(c + 1) * VS]
        m = dict(common)
        m["ids"] = input_ids[c * T:(c + 1) * T]
        m["pos"] = np.ascontiguousarray(pos_w[s0:s0 + T])
        m["lm_wT"] = np.ascontiguousarray(lm_shard.T).astype(bf)
        m["lm_b"] = lmb_shard
        in_maps.append(m)
    return in_maps


def _assemble(results):
    parts = [results[c]["out_tok"][:, :VS] for c in range(NCORES)]
    logits = np.concatenate(parts, axis=1)     # [2048, 32000]
    return np.ascontiguousarray(logits.reshape(B, S, V).astype(np.float32))


_NC_CACHE = {}


def _get_nc(n_layers=L):
    if n_layers not in _NC_CACHE:
        _NC_CACHE[n_layers] = build(n_layers)
    return _NC_CACHE[n_layers]


def run(inputs, n_layers=L, trace=False, trace_cores=None):
    nc = _get_nc(n_layers)
    in_maps = _prep_in_maps(inputs, n_layers)
    if trace:
        try:
            import axon_ntff_shim
            axon_ntff_shim.install()
        except Exception:
            pass
    res = bass_utils.run_bass_kernel_spmd(
        nc, in_maps, core_ids=list(range(NCORES)), trace=trace,
        trace_cores=(trace_cores or [0]) if trace else None)
    return _assemble(res.results), res


def kernel(**inputs) -> np.ndarray:
    out, _ = run(inputs)
    return out
